# revision 10
# baseline (speedup 1.0000x reference)
"""Trainium2 Bass kernel for nn_Conv2dModulated (modulated transposed conv + blur).

Math restructure (validated vs reference to 5e-7 rel in fp32):
  s = w @ affine_w.T + affine_b + 1                    (B, CIN)  host
  d = rsqrt(s^2 @ sum_kk(W^2).T + 1e-8)               (B, COUT) host
  out[b] = d[b,:]/16 * blur(convT2x(s[b,:] * x[b], W)) + bias
- Modulation folds into x (per-input-channel scale), demodulation into the
  PSUM eviction (per-output-channel scale) -> weights stay sample-independent.
- Stride-2 transposed conv = 4 parity classes of <=2x2-tap convs on the 32x32
  input (subpixel decomposition; 9 effective taps instead of 36 dilated).
- Blur [1,3,3,1]^2/16 = three [1,1] passes per dim. Conv output is kept
  column-parity-split: planes E/O of the zero-padded 67-col grid, stored as
  FLAT [67*34] bf16 rows so every DVE op is one contiguous run (2x mode, no
  per-row bubbles). Shifted operands are SBUF->SBUF DMA copies (free).

Schedule (evolved from the 187us baseline):
- Weights are oc-major [NCO][NCI][P, 9*P] so round 0 gates on just
  wt[0,0]+x(0,0) (~0.6 MB) instead of the whole 5 MB weight block; PE
  warm-up matmuls on a zeroed tile ramp the P-state during the DMA head.
- s1O / c1O blur adds run on Pool (gpsimd) to keep DVE under the 17.3us
  round period (DVE was co-bottleneck with PE at 137us).
- The LAST round's eviction+blur is pipelined in 3 row-chunks so most of
  its blur overlaps its own matmuls (the tail was 41us of trailing DVE).

Sharding: data-parallel over batch, 2 samples per core, 8 cores, no
collectives.
"""

import os
from contextlib import ExitStack

import numpy as np
import ml_dtypes

import concourse.bass as bass
import concourse.tile as tile
from concourse import mybir
from concourse.bass_utils import run_bass_kernel_spmd

B, CIN, COUT, LAT, H, W_SP, KK = 16, 512, 512, 512, 32, 32, 3
NCORES = 8
BPC = B // NCORES  # samples per core
P = 128
NCI = CIN // P
NCO = COUT // P
BF16 = mybir.dt.bfloat16
F32 = mybir.dt.float32
PW = 34          # plane width (67-col padded grid split by col parity)
PL = 67 * PW     # plane flat length (2278)

POOL_OFFLOAD = True   # run s1O / c1O adds on Pool instead of DVE

_ENG_PREFIX = {
    "PE": "PE_", "DVE": "DVE_", "Activation": "Activation_",
    "Pool": "Pool_", "SP": "SP_",
}


def _fix_waits(nc: bass.Bass) -> None:
    """Walrus codegen accepts only one sem-wait per compute instruction;
    Tile emits up to 4.

    1) Drop same-engine self-waits: every engine executes its stream
       serially in order (PE matmul completion is pc-monotone; DVE/ACT
       have a hardware output-drain between ops), so a wait on the
       engine's own completion semaphore is redundant.
    2) Split any remaining multi-wait onto same-engine NoOp instructions
       inserted just before the instruction.
    """
    for f in nc.m.functions:
        for bb in f.blocks:
            out = []
            for inst in bb.instructions:
                si = inst.sync_info
                if si is None or len(si.on_wait) <= 1:
                    out.append(inst)
                    continue
                eng = str(inst.engine).split(".")[-1]
                pfx = _ENG_PREFIX.get(eng)
                waits = list(si.on_wait)
                keep = [
                    w for w in waits
                    if not (pfx and (w.ant_name or "").startswith(pfx))
                ]
                for w in keep[:-1]:
                    nop = mybir.InstNoOp(name=nc.get_next_instruction_name())
                    nop.engine = inst.engine
                    nop.sync_info = mybir.SyncInfo(on_wait=[w], on_update=[])
                    out.append(nop)
                keep = keep[-1:]
                inst.sync_info = mybir.SyncInfo(
                    on_wait=keep, on_update=list(si.on_update)
                )
                out.append(inst)
            bb.instructions = out


# Parity-class geometry: (eh, ec) -> row taps, col taps, ncols.
# Output padded position: row 1+eh+2u, col 1+ec+2v.
_RTAPS = {0: [(0, 0), (2, 1)], 1: [(1, 1)]}
_CTAPS = {0: [(0, 0), (2, 1)], 1: [(1, 1)]}
# u-chunks per eh for the 3-row-chunk pipelined round
# (padded-row chunks A:1..24, B:25..46, C:47..65/66)
_UCHUNKS = {0: [(0, 12), (12, 11), (23, 10)],   # rows 1+2u
            1: [(0, 12), (12, 11), (23, 9)]}    # rows 2+2u
# H-blur c1/c2/c3 row ranges (inclusive) per chunk
_HROWS = [  # (c1a, c1b, c2a, c2b, c3a, c3b)
    (0, 23, 0, 22, 0, 21),
    (24, 45, 23, 44, 22, 43),
    (46, 65, 45, 64, 44, 63),
]


def build_program() -> bass.Bass:
    nc = bass.Bass()
    xp_d = nc.declare_dram_parameter("xp", [BPC, NCI, P, 34 * 34], BF16, isOutput=False)
    wt_d = nc.declare_dram_parameter("wt", [NCO, NCI, P, 9 * P], BF16, isOutput=False)
    dsc_d = nc.declare_dram_parameter("dsc", [P, BPC * NCO], F32, isOutput=False)
    bsc_d = nc.declare_dram_parameter("bsc", [P, NCO], F32, isOutput=False)
    out_d = nc.declare_dram_parameter("out", [BPC, NCO, P, 64 * 64], F32, isOutput=True)

    with ExitStack() as ctx:
        tc = ctx.enter_context(tile.TileContext(nc))
        consts = ctx.enter_context(tc.tile_pool(name="consts", bufs=1))
        xpool = ctx.enter_context(tc.tile_pool(name="xpool", bufs=1))
        psum = ctx.enter_context(tc.tile_pool(name="psum", bufs=8, space="PSUM"))
        spool = ctx.enter_context(tc.tile_pool(name="spool", bufs=2))
        spool1 = ctx.enter_context(tc.tile_pool(name="spool1", bufs=1))
        opool = ctx.enter_context(tc.tile_pool(name="opool", bufs=2))

        w_sb = consts.tile([P, NCO, NCI, 9 * P], BF16, tag="wsb")
        d_sb = consts.tile([P, BPC * NCO], F32, tag="dsb")
        b_sb = consts.tile([P, NCO], F32, tag="bsb")
        x_tiles = {}

        def load_x(s, c):
            t = xpool.tile([P, 34, 34], BF16, tag=f"x{s}{c}", name=f"x{s}{c}")
            nc.sync.dma_start(
                out=t[:], in_=xp_d[s, c].rearrange("p (a b) -> p a b", b=34)
            )
            x_tiles[(s, c)] = t

        # DMA issue order = need order: scales (tiny), then round-0 gate
        # (wt[0,c] + x(0,c) interleaved), then remaining weights, then s=1 x.
        nc.sync.dma_start(out=d_sb[:], in_=dsc_d[:])
        nc.sync.dma_start(out=b_sb[:], in_=bsc_d[:])
        for c in range(NCI):
            nc.sync.dma_start(out=w_sb[:, 0, c, :], in_=wt_d[0, c])
            load_x(0, c)
        for c in range(NCI):
            nc.sync.dma_start(out=w_sb[:, 1, c, :], in_=wt_d[1, c])
        for c in range(NCI):
            load_x(1, c)
        for oc in (2, 3):
            for c in range(NCI):
                nc.sync.dma_start(out=w_sb[:, oc, c, :], in_=wt_d[oc, c])

        # PE warm-up on a zeroed tile: ramps the P-state during the DMA
        # head so real matmuls start at full clock. The memzero is FIRST
        # on ACT (no DMA dependency) so the warm-up starts immediately.
        wz = consts.tile([P, 512], BF16, tag="wz")
        nc.scalar.memzero(wz[:])
        warm_ps = [psum.tile([P, 512], F32, tag="ps", name=f"wps{i}") for i in range(2)]
        for i in range(10):
            nc.tensor.matmul(
                warm_ps[i % 2][:], wz[:, 0:P], wz[:], start=True, stop=True)

        # Persistent column-parity planes of the zero-padded 67x67 grid,
        # stored flat ([67*34] + one pad row so shifted reads stay in
        # bounds). yE col m <-> padded col 2m ; yO col m <-> padded col
        # 2m+1 (col 33 = pad). Zeroed once; borders/pads stay zero,
        # interiors are fully overwritten by every eviction round.
        plane_sets = []
        for i in range(2):
            ye = consts.tile([P, PL + PW], BF16, tag=f"ye{i}")
            yo = consts.tile([P, PL + PW], BF16, tag=f"yo{i}")
            for t in (ye, yo):
                nc.scalar.memzero(t[:])
            plane_sets.append((ye, yo))

        # Engine warm-up ops that absorb DMA-completion waits, so downstream
        # compute instructions stay within the 2-sem-wait ISA limit.
        warm_a = consts.tile([P, 1], F32, tag="warm_a")
        nc.scalar.copy(warm_a[:], d_sb[:, 0:1])
        warm_v = consts.tile([P, 1], F32, tag="warm_v")
        nc.vector.tensor_copy(warm_v[:], b_sb[:, 0:1])

        NR = BPC * NCO  # 8 rounds

        def emit_round_full(s, oc, rnd):
            """Rounds 0..NR-2: c-outer matmuls, full-plane blur."""
            yE, yO = plane_sets[rnd % 2]
            for eh, ec in ((0, 0), (0, 1), (1, 0), (1, 1)):
                rtaps, ctaps = _RTAPS[eh], _CTAPS[ec]
                ncols = 33 if ec == 0 else 32
                if eh == 0:
                    rchunks = [(0, 11), (11, 11), (22, 11)]
                elif ec == 0:
                    rchunks = [(0, 11), (11, 11), (22, 10)]
                else:
                    rchunks = [(0, 16), (16, 16)]
                taps = [(kh, kw, ra, cb) for (kh, ra) in rtaps for (kw, cb) in ctaps]
                ptiles = [
                    psum.tile([P, 512], F32, tag="ps", name=f"ps{s}{oc}{eh}{ec}{fc}")
                    for fc in range(len(rchunks))
                ]
                nmm = len(taps) * NCI
                i = 0
                for c in range(NCI):          # c-outer: chunk-0 DMAs gate less
                    for kh, kw, ra, cb in taps:
                        lhsT = w_sb[:, oc, c, (kh * 3 + kw) * P : (kh * 3 + kw + 1) * P]
                        for fc, (u0, nr) in enumerate(rchunks):
                            rhs = x_tiles[(s, c)][:, u0 + ra : u0 + ra + nr,
                                                  cb : cb + ncols]
                            nc.tensor.matmul(
                                ptiles[fc][:, : nr * ncols], lhsT, rhs,
                                start=(i == 0), stop=(i == nmm - 1),
                            )
                        i += 1
                # evict into the parity plane: padded row 1+eh+2u,
                # padded col 1+ec+2v -> ec=0: yO col v ; ec=1: yE col v+1
                plane = yO if ec == 0 else yE
                col0 = 0 if ec == 0 else 1
                pv = plane[:, 0:PL].rearrange("p (r c) -> p r c", c=PW)
                for fc, (u0, nr) in enumerate(rchunks):
                    src = ptiles[fc][:, : nr * ncols].rearrange(
                        "p (r c) -> p r c", c=ncols
                    )
                    rsl = slice(1 + eh + 2 * u0, 1 + eh + 2 * (u0 + nr), 2)
                    nc.scalar.activation(
                        pv[:, rsl, col0 : col0 + ncols], src,
                        mybir.ActivationFunctionType.Copy,
                        bias=0.0,
                        scale=d_sb[:, rnd : rnd + 1],
                    )

            # --- W blur: three [1,1] passes per output col parity.
            s1E = spool.tile([P, PL], BF16, tag="s1E", name=f"s1E{rnd}")
            s1O = spool.tile([P, PL], BF16, tag="s1O", name=f"s1O{rnd}")
            s2E = spool.tile([P, PL], BF16, tag="s2E", name=f"s2E{rnd}")
            s2O = spool.tile([P, PL], BF16, tag="s2O", name=f"s2O{rnd}")
            zzE = spool.tile([P, PL], BF16, tag="zzE", name=f"zzE{rnd}")
            zzO = spool.tile([P, PL], BF16, tag="zzO", name=f"zzO{rnd}")
            yEs = spool1.tile([P, PL], BF16, tag="yEs", name=f"yEs{rnd}")
            s1Es = spool1.tile([P, PL], BF16, tag="s1Es", name=f"s1Es{rnd}")
            s2Es = spool1.tile([P, PL], BF16, tag="s2Es", name=f"s2Es{rnd}")
            nc.sync.dma_start(out=yEs[:], in_=yE[:, 1 : PL + 1])
            nc.vector.tensor_add(s1E[:], yE[:, 0:PL], yO[:, 0:PL])
            nc.vector.tensor_add(s1O[:], yO[:, 0:PL], yEs[:])
            nc.sync.dma_start(out=s1Es[:, 0 : PL - 1], in_=s1E[:, 1:PL])
            nc.vector.tensor_add(s2E[:], s1E[:], s1O[:])
            nc.vector.tensor_add(s2O[:], s1O[:], s1Es[:])
            nc.sync.dma_start(out=s2Es[:, 0 : PL - 1], in_=s2E[:, 1:PL])
            nc.vector.tensor_add(zzE[:], s2E[:], s2O[:])
            nc.vector.tensor_add(zzO[:], s2O[:], s2Es[:])

            # --- H blur per plane: three flat row-shifted passes.
            of = opool.tile([P, 64, 64], F32, tag="out", name=f"of{rnd}")
            for pw_, zp, t1, t2, t3 in (
                (0, zzE, "s1E", "s2E", "zzE"),
                (1, zzO, "s1O", "s2O", "zzO"),
            ):
                # O-side c2/c3 run on Pool: they are chain-terminal (only
                # the ACT interleave consumes them), so the slow Pool ops
                # never stall DVE. Pool ~4.7us/plane-add, 2/round.
                tail_eng = nc.gpsimd if (POOL_OFFLOAD and pw_ == 1) else nc.vector
                c1 = spool.tile([P, PL], BF16, tag=t1, name=f"c1_{rnd}{pw_}")
                nc.vector.tensor_add(
                    c1[:, 0 : 66 * PW], zp[:, 0 : 66 * PW], zp[:, PW : PL])
                c2 = spool.tile([P, PL], BF16, tag=t2, name=f"c2_{rnd}{pw_}")
                tail_eng.tensor_add(
                    c2[:, 0 : 65 * PW], c1[:, 0 : 65 * PW], c1[:, PW : 66 * PW])
                c3 = spool.tile([P, PL], BF16, tag=t3, name=f"c3_{rnd}{pw_}")
                tail_eng.tensor_add(
                    c3[:, 0 : 64 * PW], c2[:, 0 : 64 * PW], c2[:, PW : 65 * PW])
                # col-interleave + bias + fp32 convert, split in row halves
                # so the out-DMA can start early. The O side stays on Pool
                # (ACT's in-order queue must never wait on a Pool op, else
                # the next round's evictions block behind it and PE stalls
                # on PSUM).
                c3v = c3[:, 0 : 64 * PW].rearrange("p (r c) -> p r c", c=PW)
                for rh in (0, 1):
                    if POOL_OFFLOAD and pw_ == 1:
                        nc.gpsimd.tensor_scalar_add(
                            of[:, 32 * rh : 32 * (rh + 1), pw_ : 64 : 2],
                            c3v[:, 32 * rh : 32 * (rh + 1), 0:32],
                            b_sb[:, oc : oc + 1],
                        )
                    else:
                        nc.scalar.activation(
                            of[:, 32 * rh : 32 * (rh + 1), pw_ : 64 : 2],
                            c3v[:, 32 * rh : 32 * (rh + 1), 0:32],
                            mybir.ActivationFunctionType.Identity,
                            bias=b_sb[:, oc : oc + 1], scale=1.0,
                        )
            for rh in (0, 1):
                nc.sync.dma_start(
                    out=out_d[s, oc, :, 2048 * rh : 2048 * (rh + 1)],
                    in_=of[:, 32 * rh : 32 * (rh + 1), :].rearrange(
                        "p a b -> p (a b)"),
                )

        def emit_round_chunked(s, oc, rnd):
            """Last round: 3 row-chunks, blur pipelined into the matmul
            stream so only ~1/3 of the blur trails the PE."""
            yE, yO = plane_sets[rnd % 2]
            pv = {}
            for plane, key in ((yO, 0), (yE, 1)):   # key = ec
                pv[key] = plane[:, 0:PL].rearrange("p (r c) -> p r c", c=PW)

            s1E = spool.tile([P, PL], BF16, tag="s1E", name=f"s1E{rnd}")
            s1O = spool.tile([P, PL], BF16, tag="s1O", name=f"s1O{rnd}")
            s2E = spool.tile([P, PL], BF16, tag="s2E", name=f"s2E{rnd}")
            s2O = spool.tile([P, PL], BF16, tag="s2O", name=f"s2O{rnd}")
            zzE = spool.tile([P, PL], BF16, tag="zzE", name=f"zzE{rnd}")
            zzO = spool.tile([P, PL], BF16, tag="zzO", name=f"zzO{rnd}")
            yEs = spool1.tile([P, PL], BF16, tag="yEs", name=f"yEs{rnd}")
            s1Es = spool1.tile([P, PL], BF16, tag="s1Es", name=f"s1Es{rnd}")
            s2Es = spool1.tile([P, PL], BF16, tag="s2Es", name=f"s2Es{rnd}")
            c1E = spool.tile([P, PL], BF16, tag="s1E", name=f"c1E{rnd}")
            c1O = spool.tile([P, PL], BF16, tag="s1O", name=f"c1O{rnd}")
            c2E = spool.tile([P, PL], BF16, tag="s2E", name=f"c2E{rnd}")
            c2O = spool.tile([P, PL], BF16, tag="s2O", name=f"c2O{rnd}")
            c3E = spool.tile([P, PL], BF16, tag="zzE", name=f"c3E{rnd}")
            c3O = spool.tile([P, PL], BF16, tag="zzO", name=f"c3O{rnd}")
            of = opool.tile([P, 64, 64], F32, tag="out", name=f"of{rnd}")

            # W-chunk flat row ranges (rows of the 67-row padded grid,
            # chunk A includes pad row 0, chunk C pad row 66)
            wrows = [(0, 25), (25, 47), (47, 67)]

            for ck in range(3):
                # --- matmuls for this chunk, all 4 parity classes
                ptiles = {}
                for eh, ec in ((0, 0), (0, 1), (1, 0), (1, 1)):
                    ptiles[(eh, ec)] = psum.tile(
                        [P, 512], F32, tag="ps", name=f"psc{ck}{eh}{ec}")
                cnt = {}
                tot = {}
                for eh, ec in ptiles:
                    tot[(eh, ec)] = len(_RTAPS[eh]) * len(_CTAPS[ec]) * NCI
                    cnt[(eh, ec)] = 0
                for c in range(NCI):
                    for eh, ec in ((0, 0), (0, 1), (1, 0), (1, 1)):
                        u0, nr = _UCHUNKS[eh][ck]
                        ncols = 33 if ec == 0 else 32
                        for kh, ra in _RTAPS[eh]:
                            for kw, cb in _CTAPS[ec]:
                                lhsT = w_sb[:, oc, c,
                                            (kh * 3 + kw) * P : (kh * 3 + kw + 1) * P]
                                rhs = x_tiles[(s, c)][:, u0 + ra : u0 + ra + nr,
                                                      cb : cb + ncols]
                                i = cnt[(eh, ec)]
                                nc.tensor.matmul(
                                    ptiles[(eh, ec)][:, : nr * ncols], lhsT, rhs,
                                    start=(i == 0), stop=(i == tot[(eh, ec)] - 1),
                                )
                                cnt[(eh, ec)] += 1
                # --- evict chunk
                for eh, ec in ((0, 0), (0, 1), (1, 0), (1, 1)):
                    u0, nr = _UCHUNKS[eh][ck]
                    ncols = 33 if ec == 0 else 32
                    col0 = 0 if ec == 0 else 1
                    src = ptiles[(eh, ec)][:, : nr * ncols].rearrange(
                        "p (r c) -> p r c", c=ncols)
                    rsl = slice(1 + eh + 2 * u0, 1 + eh + 2 * (u0 + nr), 2)
                    nc.scalar.activation(
                        pv[ec][:, rsl, col0 : col0 + ncols], src,
                        mybir.ActivationFunctionType.Copy,
                        bias=0.0,
                        scale=d_sb[:, rnd : rnd + 1],
                    )
                # --- W blur for this chunk's rows
                r0, r1 = wrows[ck]
                a, b_ = r0 * PW, r1 * PW
                # s1E/s2E are [P, PL] tiles: clamp the +1-shifted source to
                # PL for the last chunk. The one missing tail element only
                # feeds pad col 33 of the O plane, never read downstream.
                e = min(b_ + 1, PL)
                nc.sync.dma_start(out=yEs[:, a:b_], in_=yE[:, a + 1 : b_ + 1])
                nc.vector.tensor_add(s1E[:, a:b_], yE[:, a:b_], yO[:, a:b_])
                nc.vector.tensor_add(s1O[:, a:b_], yO[:, a:b_], yEs[:, a:b_])
                nc.sync.dma_start(out=s1Es[:, a : e - 1], in_=s1E[:, a + 1 : e])
                nc.vector.tensor_add(s2E[:, a:b_], s1E[:, a:b_], s1O[:, a:b_])
                nc.vector.tensor_add(s2O[:, a:b_], s1O[:, a:b_], s1Es[:, a:b_])
                nc.sync.dma_start(out=s2Es[:, a : e - 1], in_=s2E[:, a + 1 : e])
                nc.vector.tensor_add(zzE[:, a:b_], s2E[:, a:b_], s2O[:, a:b_])
                nc.vector.tensor_add(zzO[:, a:b_], s2O[:, a:b_], s2Es[:, a:b_])
                # --- H blur + interleave + out DMA for this chunk
                c1a, c1b, c2a, c2b, c3a, c3b = _HROWS[ck]
                for zp, c1, c2, c3 in ((zzE, c1E, c2E, c3E), (zzO, c1O, c2O, c3O)):
                    nc.vector.tensor_add(
                        c1[:, c1a * PW : (c1b + 1) * PW],
                        zp[:, c1a * PW : (c1b + 1) * PW],
                        zp[:, (c1a + 1) * PW : (c1b + 2) * PW])
                    nc.vector.tensor_add(
                        c2[:, c2a * PW : (c2b + 1) * PW],
                        c1[:, c2a * PW : (c2b + 1) * PW],
                        c1[:, (c2a + 1) * PW : (c2b + 2) * PW])
                    nc.vector.tensor_add(
                        c3[:, c3a * PW : (c3b + 1) * PW],
                        c2[:, c3a * PW : (c3b + 1) * PW],
                        c2[:, (c3a + 1) * PW : (c3b + 2) * PW])
                for pw_, c3 in ((0, c3E), (1, c3O)):
                    c3v = c3[:, 0 : 64 * PW].rearrange("p (r c) -> p r c", c=PW)
                    nc.scalar.activation(
                        of[:, c3a : c3b + 1, pw_ : 64 : 2],
                        c3v[:, c3a : c3b + 1, 0:32],
                        mybir.ActivationFunctionType.Identity,
                        bias=b_sb[:, oc : oc + 1], scale=1.0,
                    )
                nc.sync.dma_start(
                    out=out_d[s, oc, :, 64 * c3a : 64 * (c3b + 1)],
                    in_=of[:, c3a : c3b + 1, :].rearrange("p a b -> p (a b)"),
                )

        for s in range(BPC):
            for oc in range(NCO):
                rnd = s * NCO + oc
                if s == 0 and oc >= 1:
                    # absorb wt[oc,*] DMA sems before the round needs them
                    for c in range(NCI):
                        pwm = psum.tile([P, 512], F32, tag="ps", name=f"pswt{oc}{c}")
                        nc.tensor.matmul(
                            pwm[:, :16], w_sb[:, oc, c, 0:P],
                            x_tiles[(0, c)][:, 0, 0:16],
                            start=True, stop=True,
                        )
                if rnd == 4:
                    # absorb the x(1,*) DMA sems before s=1 rounds
                    for c in range(NCI):
                        pwm = psum.tile([P, 512], F32, tag="ps", name=f"pswm{c}")
                        nc.tensor.matmul(
                            pwm[:, :16], w_sb[:, 0, c, 0:P],
                            x_tiles[(1, c)][:, 0, 0:16],
                            start=True, stop=True,
                        )
                if rnd == NR - 1:
                    emit_round_chunked(s, oc, rnd)
                else:
                    emit_round_full(s, oc, rnd)
    _fix_waits(nc)
    return nc


def make_in_maps(x, w, weight, bias, affine_w, affine_b):
    x = np.asarray(x, np.float32)
    w = np.asarray(w, np.float32)
    weight = np.asarray(weight, np.float32)
    bias = np.asarray(bias, np.float32)
    affine_w = np.asarray(affine_w, np.float32)
    affine_b = np.asarray(affine_b, np.float32)

    s = w @ affine_w.T + affine_b + 1.0  # (B, CIN)
    wsq = (weight.astype(np.float64) ** 2).sum(axis=(2, 3))  # (COUT, CIN)
    d = 1.0 / np.sqrt((s.astype(np.float64) ** 2) @ wsq.T + 1e-8)  # (B, COUT)
    d16 = (d / 16.0).astype(np.float32)

    xp = np.zeros((B, CIN, 34, 34), np.float32)
    xp[:, :, 1:33, 1:33] = x * s[:, :, None, None]
    xp_bf = xp.astype(ml_dtypes.bfloat16).reshape(B, NCI, P, 34 * 34)

    wf = weight[:, :, ::-1, ::-1]  # spatial flip
    # oc-major layout: wt[oc, c, p, (kh*3+kw)*P + m] = wf[oc*P+m, c*P+p, kh, kw]
    wt = np.ascontiguousarray(
        wf.transpose(1, 2, 3, 0)                    # (CIN, 3, 3, COUT)
        .reshape(NCI, P, 9, NCO, P)
        .transpose(3, 0, 1, 2, 4)                   # (NCO, NCI, P, 9, P)
        .reshape(NCO, NCI, P, 9 * P)
    ).astype(ml_dtypes.bfloat16)

    bsc = np.ascontiguousarray(bias.reshape(COUT).reshape(NCO, P).T).astype(np.float32)

    in_maps = []
    for core in range(NCORES):
        sl = slice(core * BPC, (core + 1) * BPC)
        dcore = d16[sl].reshape(BPC, NCO, P)
        dsc = np.ascontiguousarray(dcore.transpose(2, 0, 1).reshape(P, BPC * NCO))
        in_maps.append(
            {
                "xp": np.ascontiguousarray(xp_bf[sl]),
                "wt": wt,
                "dsc": dsc,
                "bsc": bsc,
            }
        )
    return in_maps


LAST_RESULTS = None  # BassKernelResults of the most recent run (for test harness)


def kernel(x, w, weight, bias, affine_w, affine_b):
    global LAST_RESULTS
    in_maps = make_in_maps(x, w, weight, bias, affine_w, affine_b)
    nc = build_program()
    res = run_bass_kernel_spmd(nc, in_maps, list(range(NCORES)))
    LAST_RESULTS = res
    outs = [r["out"].reshape(BPC, COUT, 64, 64) for r in res.results]
    return np.ascontiguousarray(np.concatenate(outs, axis=0), dtype=np.float32)


# revision 14
# speedup vs baseline: 2.1207x; 2.1207x over previous
"""Trainium2 Bass kernel for nn_Conv2dModulated (modulated transposed conv + blur).

Math restructure (validated vs reference to 5e-7 rel in fp32):
  s = w @ affine_w.T + affine_b + 1                    (B, CIN)  host
  d = rsqrt(s^2 @ sum_kk(W^2).T + 1e-8)               (B, COUT) host
  out[b] = d[b,:]/16 * blur(convT2x(s[b,:] * x[b], W)) + bias
- Modulation folds into x (per-input-channel scale), demodulation into the
  PSUM eviction (per-output-channel scale) -> weights stay sample-independent.
- Stride-2 transposed conv = 4 parity classes of <=2x2-tap convs on the 32x32
  input (subpixel decomposition; 9 effective taps instead of 36 dilated).
- Blur [1,3,3,1]^2/16 = three [1,1] passes per dim. Conv output is kept
  column-parity-split: planes E/O of the zero-padded 67-col grid, stored as
  FLAT [67*34] bf16 rows so every DVE op is one contiguous run (2x mode, no
  per-row bubbles). Shifted operands are SBUF->SBUF DMA copies (free).

Schedule (evolved from the 187us baseline):
- Weights are oc-major [NCO][NCI][P, 9*P] so round 0 gates on just
  wt[0,0]+x(0,0) (~0.6 MB) instead of the whole 5 MB weight block; PE
  warm-up matmuls on a zeroed tile ramp the P-state during the DMA head.
- s1O / c1O blur adds run on Pool (gpsimd) to keep DVE under the 17.3us
  round period (DVE was co-bottleneck with PE at 137us).
- The LAST round's eviction+blur is pipelined in 3 row-chunks so most of
  its blur overlaps its own matmuls (the tail was 41us of trailing DVE).

Sharding: data-parallel over batch, 2 samples per core, 8 cores, no
collectives.
"""

import os
from contextlib import ExitStack

import numpy as np
import ml_dtypes

import concourse.bass as bass
import concourse.tile as tile
from concourse import mybir
from concourse.bass_utils import run_bass_kernel_spmd

B, CIN, COUT, LAT, H, W_SP, KK = 16, 512, 512, 512, 32, 32, 3
NCORES = 8
BPC = B // NCORES  # samples per core
P = 128
NCI = CIN // P
NCO = COUT // P
BF16 = mybir.dt.bfloat16
F32 = mybir.dt.float32
PW = 34          # plane width (67-col padded grid split by col parity)
PL = 67 * PW     # plane flat length (2278)

POOL_OFFLOAD = True   # run s1O / c1O adds on Pool instead of DVE

_ENG_PREFIX = {
    "PE": "PE_", "DVE": "DVE_", "Activation": "Activation_",
    "Pool": "Pool_", "SP": "SP_",
}


def _fix_waits(nc: bass.Bass) -> None:
    """Walrus codegen accepts only one sem-wait per compute instruction;
    Tile emits up to 4.

    1) Drop same-engine self-waits: every engine executes its stream
       serially in order (PE matmul completion is pc-monotone; DVE/ACT
       have a hardware output-drain between ops), so a wait on the
       engine's own completion semaphore is redundant.
    2) Split any remaining multi-wait onto same-engine NoOp instructions
       inserted just before the instruction.
    """
    for f in nc.m.functions:
        for bb in f.blocks:
            out = []
            for inst in bb.instructions:
                si = inst.sync_info
                if si is None or len(si.on_wait) <= 1:
                    out.append(inst)
                    continue
                eng = str(inst.engine).split(".")[-1]
                pfx = _ENG_PREFIX.get(eng)
                waits = list(si.on_wait)
                keep = [
                    w for w in waits
                    if not (pfx and (w.ant_name or "").startswith(pfx))
                ]
                for w in keep[:-1]:
                    nop = mybir.InstNoOp(name=nc.get_next_instruction_name())
                    nop.engine = inst.engine
                    nop.sync_info = mybir.SyncInfo(on_wait=[w], on_update=[])
                    out.append(nop)
                keep = keep[-1:]
                inst.sync_info = mybir.SyncInfo(
                    on_wait=keep, on_update=list(si.on_update)
                )
                out.append(inst)
            bb.instructions = out


# Parity-class geometry: (eh, ec) -> row taps, col taps, ncols.
# Output padded position: row 1+eh+2u, col 1+ec+2v.
_RTAPS = {0: [(0, 0), (2, 1)], 1: [(1, 1)]}
_CTAPS = {0: [(0, 0), (2, 1)], 1: [(1, 1)]}
# u-chunks per eh for the 3-row-chunk pipelined round
# (padded-row chunks A:1..24, B:25..46, C:47..65/66)
_UCHUNKS = {0: [(0, 12), (12, 11), (23, 10)],   # rows 1+2u
            1: [(0, 12), (12, 11), (23, 9)]}    # rows 2+2u
# H-blur c1/c2/c3 row ranges (inclusive) per chunk
_HROWS = [  # (c1a, c1b, c2a, c2b, c3a, c3b)
    (0, 23, 0, 22, 0, 21),
    (24, 45, 23, 44, 22, 43),
    (46, 65, 45, 64, 44, 63),
]


def build_program() -> bass.Bass:
    nc = bass.Bass()
    xp_d = nc.declare_dram_parameter("xp", [BPC, NCI, P, 34 * 34], BF16, isOutput=False)
    wt_d = nc.declare_dram_parameter("wt", [NCO, NCI, P, 9 * P], BF16, isOutput=False)
    dsc_d = nc.declare_dram_parameter("dsc", [P, BPC * NCO], F32, isOutput=False)
    bsc_d = nc.declare_dram_parameter("bsc", [P, NCO], F32, isOutput=False)
    out_d = nc.declare_dram_parameter("out", [BPC, NCO, P, 64 * 64], F32, isOutput=True)

    with ExitStack() as ctx:
        tc = ctx.enter_context(tile.TileContext(nc))
        consts = ctx.enter_context(tc.tile_pool(name="consts", bufs=1))
        xpool = ctx.enter_context(tc.tile_pool(name="xpool", bufs=1))
        psum = ctx.enter_context(tc.tile_pool(name="psum", bufs=8, space="PSUM"))
        spool = ctx.enter_context(tc.tile_pool(name="spool", bufs=2))
        spool1 = ctx.enter_context(tc.tile_pool(name="spool1", bufs=1))
        opool = ctx.enter_context(tc.tile_pool(name="opool", bufs=2))

        w_sb = consts.tile([P, NCO, NCI, 9 * P], BF16, tag="wsb")
        d_sb = consts.tile([P, BPC * NCO], F32, tag="dsb")
        b_sb = consts.tile([P, NCO], F32, tag="bsb")
        x_tiles = {}

        def load_x(s, c):
            t = xpool.tile([P, 34, 34], BF16, tag=f"x{s}{c}", name=f"x{s}{c}")
            nc.sync.dma_start(
                out=t[:], in_=xp_d[s, c].rearrange("p (a b) -> p a b", b=34)
            )
            x_tiles[(s, c)] = t

        # DMA issue order = need order: scales (tiny), then round-0 gate
        # (wt[0,c] + x(0,c) interleaved), then remaining weights, then s=1 x.
        nc.sync.dma_start(out=d_sb[:], in_=dsc_d[:])
        nc.sync.dma_start(out=b_sb[:], in_=bsc_d[:])
        for c in range(NCI):
            nc.sync.dma_start(out=w_sb[:, 0, c, :], in_=wt_d[0, c])
            load_x(0, c)
        for c in range(NCI):
            nc.sync.dma_start(out=w_sb[:, 1, c, :], in_=wt_d[1, c])
        for c in range(NCI):
            load_x(1, c)
        for oc in (2, 3):
            for c in range(NCI):
                nc.sync.dma_start(out=w_sb[:, oc, c, :], in_=wt_d[oc, c])

        # PE warm-up on a zeroed tile: ramps the P-state during the DMA
        # head so real matmuls start at full clock. The memzero is FIRST
        # on ACT (no DMA dependency) so the warm-up starts immediately.
        wz = consts.tile([P, 512], BF16, tag="wz")
        nc.scalar.memzero(wz[:])
        warm_ps = [psum.tile([P, 512], F32, tag="ps", name=f"wps{i}") for i in range(2)]
        for i in range(10):
            nc.tensor.matmul(
                warm_ps[i % 2][:], wz[:, 0:P], wz[:], start=True, stop=True)

        # Persistent column-parity planes of the zero-padded 67x67 grid,
        # stored flat ([67*34] + one pad row so shifted reads stay in
        # bounds). yE col m <-> padded col 2m ; yO col m <-> padded col
        # 2m+1 (col 33 = pad). Zeroed once; borders/pads stay zero,
        # interiors are fully overwritten by every eviction round.
        plane_sets = []
        for i in range(2):
            ye = consts.tile([P, PL + PW], BF16, tag=f"ye{i}")
            yo = consts.tile([P, PL + PW], BF16, tag=f"yo{i}")
            for t in (ye, yo):
                nc.scalar.memzero(t[:])
            plane_sets.append((ye, yo))

        # Engine warm-up ops that absorb DMA-completion waits, so downstream
        # compute instructions stay within the 2-sem-wait ISA limit.
        warm_a = consts.tile([P, 1], F32, tag="warm_a")
        nc.scalar.copy(warm_a[:], d_sb[:, 0:1])
        warm_v = consts.tile([P, 1], F32, tag="warm_v")
        nc.vector.tensor_copy(warm_v[:], b_sb[:, 0:1])

        NR = BPC * NCO  # 8 rounds
        pending = []    # deferred interleave+DMA closures (one round behind)

        def emit_round_full(s, oc, rnd):
            """Rounds 0..NR-2: c-outer matmuls, full-plane blur."""
            yE, yO = plane_sets[rnd % 2]
            for eh, ec in ((0, 0), (0, 1), (1, 0), (1, 1)):
                rtaps, ctaps = _RTAPS[eh], _CTAPS[ec]
                ncols = 33 if ec == 0 else 32
                if eh == 0:
                    rchunks = [(0, 11), (11, 11), (22, 11)]
                elif ec == 0:
                    rchunks = [(0, 11), (11, 11), (22, 10)]
                else:
                    rchunks = [(0, 16), (16, 16)]
                taps = [(kh, kw, ra, cb) for (kh, ra) in rtaps for (kw, cb) in ctaps]
                ptiles = [
                    psum.tile([P, 512], F32, tag="ps", name=f"ps{s}{oc}{eh}{ec}{fc}")
                    for fc in range(len(rchunks))
                ]
                nmm = len(taps) * NCI
                i = 0
                for c in range(NCI):          # c-outer: chunk-0 DMAs gate less
                    for kh, kw, ra, cb in taps:
                        lhsT = w_sb[:, oc, c, (kh * 3 + kw) * P : (kh * 3 + kw + 1) * P]
                        for fc, (u0, nr) in enumerate(rchunks):
                            rhs = x_tiles[(s, c)][:, u0 + ra : u0 + ra + nr,
                                                  cb : cb + ncols]
                            nc.tensor.matmul(
                                ptiles[fc][:, : nr * ncols], lhsT, rhs,
                                start=(i == 0), stop=(i == nmm - 1),
                            )
                        i += 1
                # evict into the parity plane: padded row 1+eh+2u,
                # padded col 1+ec+2v -> ec=0: yO col v ; ec=1: yE col v+1
                plane = yO if ec == 0 else yE
                col0 = 0 if ec == 0 else 1
                pv = plane[:, 0:PL].rearrange("p (r c) -> p r c", c=PW)
                for fc, (u0, nr) in enumerate(rchunks):
                    src = ptiles[fc][:, : nr * ncols].rearrange(
                        "p (r c) -> p r c", c=ncols
                    )
                    rsl = slice(1 + eh + 2 * u0, 1 + eh + 2 * (u0 + nr), 2)
                    nc.scalar.activation(
                        pv[:, rsl, col0 : col0 + ncols], src,
                        mybir.ActivationFunctionType.Copy,
                        bias=0.0,
                        scale=d_sb[:, rnd : rnd + 1],
                    )

            # previous round's interleaves: emitted here (after this
            # round's evictions) so ACT never blocks eviction work.
            for f in pending:
                f()
            pending.clear()

            # --- W blur: three [1,1] passes per output col parity.
            s1E = spool.tile([P, PL], BF16, tag="s1E", name=f"s1E{rnd}")
            s1O = spool.tile([P, PL], BF16, tag="s1O", name=f"s1O{rnd}")
            s2E = spool.tile([P, PL], BF16, tag="s2E", name=f"s2E{rnd}")
            s2O = spool.tile([P, PL], BF16, tag="s2O", name=f"s2O{rnd}")
            zzE = spool.tile([P, PL], BF16, tag="zzE", name=f"zzE{rnd}")
            zzO = spool.tile([P, PL], BF16, tag="zzO", name=f"zzO{rnd}")
            yEs = spool1.tile([P, PL], BF16, tag="yEs", name=f"yEs{rnd}")
            s1Es = spool1.tile([P, PL], BF16, tag="s1Es", name=f"s1Es{rnd}")
            s2Es = spool1.tile([P, PL], BF16, tag="s2Es", name=f"s2Es{rnd}")
            nc.sync.dma_start(out=yEs[:], in_=yE[:, 1 : PL + 1])
            nc.vector.tensor_add(s1E[:], yE[:, 0:PL], yO[:, 0:PL])
            nc.vector.tensor_add(s1O[:], yO[:, 0:PL], yEs[:])
            nc.sync.dma_start(out=s1Es[:, 0 : PL - 1], in_=s1E[:, 1:PL])
            nc.vector.tensor_add(s2E[:], s1E[:], s1O[:])
            nc.vector.tensor_add(s2O[:], s1O[:], s1Es[:])
            nc.sync.dma_start(out=s2Es[:, 0 : PL - 1], in_=s2E[:, 1:PL])
            nc.vector.tensor_add(zzE[:], s2E[:], s2O[:])
            nc.vector.tensor_add(zzO[:], s2O[:], s2Es[:])

            # --- H blur per plane: three flat row-shifted passes. O side
            # first so Pool's c2O/c3O (chain-terminal, nothing on DVE
            # waits for them) finish as early as possible.
            of = opool.tile([P, 64, 64], F32, tag="out", name=f"of{rnd}")
            c3s = {}
            for pw_, zp, t1, t2, t3 in (
                (1, zzO, "s1O", "s2O", "zzO"),
                (0, zzE, "s1E", "s2E", "zzE"),
            ):
                tail_eng = nc.gpsimd if (POOL_OFFLOAD and pw_ == 1) else nc.vector
                c1 = spool.tile([P, PL], BF16, tag=t1, name=f"c1_{rnd}{pw_}")
                nc.vector.tensor_add(
                    c1[:, 0 : 66 * PW], zp[:, 0 : 66 * PW], zp[:, PW : PL])
                c2 = spool.tile([P, PL], BF16, tag=t2, name=f"c2_{rnd}{pw_}")
                tail_eng.tensor_add(
                    c2[:, 0 : 65 * PW], c1[:, 0 : 65 * PW], c1[:, PW : 66 * PW])
                c3 = spool.tile([P, PL], BF16, tag=t3, name=f"c3_{rnd}{pw_}")
                tail_eng.tensor_add(
                    c3[:, 0 : 64 * PW], c2[:, 0 : 64 * PW], c2[:, PW : 65 * PW])
                c3s[pw_] = c3

            def do_interleave(rnd=rnd, s=s, oc=oc, of=of, c3s=c3s):
                # col-interleave + bias + fp32 convert on ACT + out DMA.
                # Deferred to after the NEXT round's evictions: the O side
                # waits on Pool's c3O, and ACT's in-order queue must not
                # block the next round's evictions behind that wait (PE
                # would stall on PSUM reuse).
                for pw_ in (0, 1):
                    c3v = c3s[pw_][:, 0 : 64 * PW].rearrange(
                        "p (r c) -> p r c", c=PW)
                    for rh in (0, 1):
                        nc.scalar.activation(
                            of[:, 32 * rh : 32 * (rh + 1), pw_ : 64 : 2],
                            c3v[:, 32 * rh : 32 * (rh + 1), 0:32],
                            mybir.ActivationFunctionType.Identity,
                            bias=b_sb[:, oc : oc + 1], scale=1.0,
                        )
                for rh in (0, 1):
                    nc.sync.dma_start(
                        out=out_d[s, oc, :, 2048 * rh : 2048 * (rh + 1)],
                        in_=of[:, 32 * rh : 32 * (rh + 1), :].rearrange(
                            "p a b -> p (a b)"),
                    )

            pending.append(do_interleave)

        def emit_round_chunked(s, oc, rnd):
            """Last round: 3 row-chunks, blur pipelined into the matmul
            stream so only ~1/3 of the blur trails the PE."""
            yE, yO = plane_sets[rnd % 2]
            pv = {}
            for plane, key in ((yO, 0), (yE, 1)):   # key = ec
                pv[key] = plane[:, 0:PL].rearrange("p (r c) -> p r c", c=PW)

            s1E = spool.tile([P, PL], BF16, tag="s1E", name=f"s1E{rnd}")
            s1O = spool.tile([P, PL], BF16, tag="s1O", name=f"s1O{rnd}")
            s2E = spool.tile([P, PL], BF16, tag="s2E", name=f"s2E{rnd}")
            s2O = spool.tile([P, PL], BF16, tag="s2O", name=f"s2O{rnd}")
            zzE = spool.tile([P, PL], BF16, tag="zzE", name=f"zzE{rnd}")
            zzO = spool.tile([P, PL], BF16, tag="zzO", name=f"zzO{rnd}")
            yEs = spool1.tile([P, PL], BF16, tag="yEs", name=f"yEs{rnd}")
            s1Es = spool1.tile([P, PL], BF16, tag="s1Es", name=f"s1Es{rnd}")
            s2Es = spool1.tile([P, PL], BF16, tag="s2Es", name=f"s2Es{rnd}")
            c1E = spool.tile([P, PL], BF16, tag="s1E", name=f"c1E{rnd}")
            c1O = spool.tile([P, PL], BF16, tag="s1O", name=f"c1O{rnd}")
            c2E = spool.tile([P, PL], BF16, tag="s2E", name=f"c2E{rnd}")
            c2O = spool.tile([P, PL], BF16, tag="s2O", name=f"c2O{rnd}")
            c3E = spool.tile([P, PL], BF16, tag="zzE", name=f"c3E{rnd}")
            c3O = spool.tile([P, PL], BF16, tag="zzO", name=f"c3O{rnd}")
            of = opool.tile([P, 64, 64], F32, tag="out", name=f"of{rnd}")

            # W-chunk flat row ranges (rows of the 67-row padded grid,
            # chunk A includes pad row 0, chunk C pad row 66)
            wrows = [(0, 25), (25, 47), (47, 67)]

            for ck in range(3):
                # --- matmuls for this chunk, all 4 parity classes
                ptiles = {}
                for eh, ec in ((0, 0), (0, 1), (1, 0), (1, 1)):
                    ptiles[(eh, ec)] = psum.tile(
                        [P, 512], F32, tag="ps", name=f"psc{ck}{eh}{ec}")
                cnt = {}
                tot = {}
                for eh, ec in ptiles:
                    tot[(eh, ec)] = len(_RTAPS[eh]) * len(_CTAPS[ec]) * NCI
                    cnt[(eh, ec)] = 0
                for c in range(NCI):
                    for eh, ec in ((0, 0), (0, 1), (1, 0), (1, 1)):
                        u0, nr = _UCHUNKS[eh][ck]
                        ncols = 33 if ec == 0 else 32
                        for kh, ra in _RTAPS[eh]:
                            for kw, cb in _CTAPS[ec]:
                                lhsT = w_sb[:, oc, c,
                                            (kh * 3 + kw) * P : (kh * 3 + kw + 1) * P]
                                rhs = x_tiles[(s, c)][:, u0 + ra : u0 + ra + nr,
                                                      cb : cb + ncols]
                                i = cnt[(eh, ec)]
                                nc.tensor.matmul(
                                    ptiles[(eh, ec)][:, : nr * ncols], lhsT, rhs,
                                    start=(i == 0), stop=(i == tot[(eh, ec)] - 1),
                                )
                                cnt[(eh, ec)] += 1
                # --- evict chunk
                for eh, ec in ((0, 0), (0, 1), (1, 0), (1, 1)):
                    u0, nr = _UCHUNKS[eh][ck]
                    ncols = 33 if ec == 0 else 32
                    col0 = 0 if ec == 0 else 1
                    src = ptiles[(eh, ec)][:, : nr * ncols].rearrange(
                        "p (r c) -> p r c", c=ncols)
                    rsl = slice(1 + eh + 2 * u0, 1 + eh + 2 * (u0 + nr), 2)
                    nc.scalar.activation(
                        pv[ec][:, rsl, col0 : col0 + ncols], src,
                        mybir.ActivationFunctionType.Copy,
                        bias=0.0,
                        scale=d_sb[:, rnd : rnd + 1],
                    )
                if ck == 1:
                    # round-6 interleaves: after chunk B's evictions its
                    # Pool-side c3O is ready, so ACT won't block chunk C.
                    for f in pending:
                        f()
                    pending.clear()
                # --- W blur for this chunk's rows
                r0, r1 = wrows[ck]
                a, b_ = r0 * PW, r1 * PW
                # s1E/s2E are [P, PL] tiles: clamp the +1-shifted source to
                # PL for the last chunk. The one missing tail element only
                # feeds pad col 33 of the O plane, never read downstream.
                e = min(b_ + 1, PL)
                nc.sync.dma_start(out=yEs[:, a:b_], in_=yE[:, a + 1 : b_ + 1])
                nc.vector.tensor_add(s1E[:, a:b_], yE[:, a:b_], yO[:, a:b_])
                nc.vector.tensor_add(s1O[:, a:b_], yO[:, a:b_], yEs[:, a:b_])
                nc.sync.dma_start(out=s1Es[:, a : e - 1], in_=s1E[:, a + 1 : e])
                nc.vector.tensor_add(s2E[:, a:b_], s1E[:, a:b_], s1O[:, a:b_])
                nc.vector.tensor_add(s2O[:, a:b_], s1O[:, a:b_], s1Es[:, a:b_])
                nc.sync.dma_start(out=s2Es[:, a : e - 1], in_=s2E[:, a + 1 : e])
                nc.vector.tensor_add(zzE[:, a:b_], s2E[:, a:b_], s2O[:, a:b_])
                nc.vector.tensor_add(zzO[:, a:b_], s2O[:, a:b_], s2Es[:, a:b_])
                # --- H blur + interleave + out DMA for this chunk
                c1a, c1b, c2a, c2b, c3a, c3b = _HROWS[ck]
                for zp, c1, c2, c3 in ((zzE, c1E, c2E, c3E), (zzO, c1O, c2O, c3O)):
                    nc.vector.tensor_add(
                        c1[:, c1a * PW : (c1b + 1) * PW],
                        zp[:, c1a * PW : (c1b + 1) * PW],
                        zp[:, (c1a + 1) * PW : (c1b + 2) * PW])
                    nc.vector.tensor_add(
                        c2[:, c2a * PW : (c2b + 1) * PW],
                        c1[:, c2a * PW : (c2b + 1) * PW],
                        c1[:, (c2a + 1) * PW : (c2b + 2) * PW])
                    nc.vector.tensor_add(
                        c3[:, c3a * PW : (c3b + 1) * PW],
                        c2[:, c3a * PW : (c3b + 1) * PW],
                        c2[:, (c3a + 1) * PW : (c3b + 2) * PW])
                for pw_, c3 in ((0, c3E), (1, c3O)):
                    c3v = c3[:, 0 : 64 * PW].rearrange("p (r c) -> p r c", c=PW)
                    nc.scalar.activation(
                        of[:, c3a : c3b + 1, pw_ : 64 : 2],
                        c3v[:, c3a : c3b + 1, 0:32],
                        mybir.ActivationFunctionType.Identity,
                        bias=b_sb[:, oc : oc + 1], scale=1.0,
                    )
                nc.sync.dma_start(
                    out=out_d[s, oc, :, 64 * c3a : 64 * (c3b + 1)],
                    in_=of[:, c3a : c3b + 1, :].rearrange("p a b -> p (a b)"),
                )

        for s in range(BPC):
            for oc in range(NCO):
                rnd = s * NCO + oc
                if s == 0 and oc >= 1:
                    # absorb wt[oc,*] DMA sems before the round needs them
                    for c in range(NCI):
                        pwm = psum.tile([P, 512], F32, tag="ps", name=f"pswt{oc}{c}")
                        nc.tensor.matmul(
                            pwm[:, :16], w_sb[:, oc, c, 0:P],
                            x_tiles[(0, c)][:, 0, 0:16],
                            start=True, stop=True,
                        )
                if rnd == 4:
                    # absorb the x(1,*) DMA sems before s=1 rounds
                    for c in range(NCI):
                        pwm = psum.tile([P, 512], F32, tag="ps", name=f"pswm{c}")
                        nc.tensor.matmul(
                            pwm[:, :16], w_sb[:, 0, c, 0:P],
                            x_tiles[(1, c)][:, 0, 0:16],
                            start=True, stop=True,
                        )
                if rnd == NR - 1:
                    emit_round_chunked(s, oc, rnd)
                else:
                    emit_round_full(s, oc, rnd)
    _fix_waits(nc)
    return nc


def make_in_maps(x, w, weight, bias, affine_w, affine_b):
    x = np.asarray(x, np.float32)
    w = np.asarray(w, np.float32)
    weight = np.asarray(weight, np.float32)
    bias = np.asarray(bias, np.float32)
    affine_w = np.asarray(affine_w, np.float32)
    affine_b = np.asarray(affine_b, np.float32)

    s = w @ affine_w.T + affine_b + 1.0  # (B, CIN)
    wsq = (weight.astype(np.float64) ** 2).sum(axis=(2, 3))  # (COUT, CIN)
    d = 1.0 / np.sqrt((s.astype(np.float64) ** 2) @ wsq.T + 1e-8)  # (B, COUT)
    d16 = (d / 16.0).astype(np.float32)

    xp = np.zeros((B, CIN, 34, 34), np.float32)
    xp[:, :, 1:33, 1:33] = x * s[:, :, None, None]
    xp_bf = xp.astype(ml_dtypes.bfloat16).reshape(B, NCI, P, 34 * 34)

    wf = weight[:, :, ::-1, ::-1]  # spatial flip
    # oc-major layout: wt[oc, c, p, (kh*3+kw)*P + m] = wf[oc*P+m, c*P+p, kh, kw]
    wt = np.ascontiguousarray(
        wf.transpose(1, 2, 3, 0)                    # (CIN, 3, 3, COUT)
        .reshape(NCI, P, 9, NCO, P)
        .transpose(3, 0, 1, 2, 4)                   # (NCO, NCI, P, 9, P)
        .reshape(NCO, NCI, P, 9 * P)
    ).astype(ml_dtypes.bfloat16)

    bsc = np.ascontiguousarray(bias.reshape(COUT).reshape(NCO, P).T).astype(np.float32)

    in_maps = []
    for core in range(NCORES):
        sl = slice(core * BPC, (core + 1) * BPC)
        dcore = d16[sl].reshape(BPC, NCO, P)
        dsc = np.ascontiguousarray(dcore.transpose(2, 0, 1).reshape(P, BPC * NCO))
        in_maps.append(
            {
                "xp": np.ascontiguousarray(xp_bf[sl]),
                "wt": wt,
                "dsc": dsc,
                "bsc": bsc,
            }
        )
    return in_maps


LAST_RESULTS = None  # BassKernelResults of the most recent run (for test harness)


def kernel(x, w, weight, bias, affine_w, affine_b):
    global LAST_RESULTS
    in_maps = make_in_maps(x, w, weight, bias, affine_w, affine_b)
    nc = build_program()
    res = run_bass_kernel_spmd(nc, in_maps, list(range(NCORES)))
    LAST_RESULTS = res
    outs = [r["out"].reshape(BPC, COUT, 64, 64) for r in res.results]
    return np.ascontiguousarray(np.concatenate(outs, axis=0), dtype=np.float32)


# revision 16
# speedup vs baseline: 2.1309x; 1.0048x over previous
"""Trainium2 Bass kernel for nn_Conv2dModulated (modulated transposed conv + blur).

Math restructure (validated vs reference to 5e-7 rel in fp32):
  s = w @ affine_w.T + affine_b + 1                    (B, CIN)  host
  d = rsqrt(s^2 @ sum_kk(W^2).T + 1e-8)               (B, COUT) host
  out[b] = d[b,:]/16 * blur(convT2x(s[b,:] * x[b], W)) + bias
- Modulation folds into x (per-input-channel scale), demodulation into the
  PSUM eviction (per-output-channel scale) -> weights stay sample-independent.
- Stride-2 transposed conv = 4 parity classes of <=2x2-tap convs on the 32x32
  input (subpixel decomposition; 9 effective taps instead of 36 dilated).
- Blur [1,3,3,1]^2/16 = three [1,1] passes per dim. Conv output is kept
  column-parity-split: planes E/O of the zero-padded 67-col grid, stored as
  FLAT [67*34] bf16 rows so every DVE op is one contiguous run (2x mode, no
  per-row bubbles). Shifted operands are SBUF->SBUF DMA copies (free).

Schedule (evolved from the 187us baseline):
- Weights are oc-major [NCO][NCI][P, 9*P] so round 0 gates on just
  wt[0,0]+x(0,0) (~0.6 MB) instead of the whole 5 MB weight block; PE
  warm-up matmuls on a zeroed tile ramp the P-state during the DMA head.
- s1O / c1O blur adds run on Pool (gpsimd) to keep DVE under the 17.3us
  round period (DVE was co-bottleneck with PE at 137us).
- The LAST round's eviction+blur is pipelined in 3 row-chunks so most of
  its blur overlaps its own matmuls (the tail was 41us of trailing DVE).

Sharding: data-parallel over batch, 2 samples per core, 8 cores, no
collectives.
"""

import os
from contextlib import ExitStack

import numpy as np
import ml_dtypes

import concourse.bass as bass
import concourse.tile as tile
from concourse import mybir
from concourse.bass_utils import run_bass_kernel_spmd

B, CIN, COUT, LAT, H, W_SP, KK = 16, 512, 512, 512, 32, 32, 3
NCORES = 8
BPC = B // NCORES  # samples per core
P = 128
NCI = CIN // P
NCO = COUT // P
BF16 = mybir.dt.bfloat16
F32 = mybir.dt.float32
PW = 34          # plane width (67-col padded grid split by col parity)
PL = 67 * PW     # plane flat length (2278)

POOL_OFFLOAD = True   # run s1O / c1O adds on Pool instead of DVE

_ENG_PREFIX = {
    "PE": "PE_", "DVE": "DVE_", "Activation": "Activation_",
    "Pool": "Pool_", "SP": "SP_",
}


def _fix_waits(nc: bass.Bass) -> None:
    """Walrus codegen accepts only one sem-wait per compute instruction;
    Tile emits up to 4.

    1) Drop same-engine self-waits: every engine executes its stream
       serially in order (PE matmul completion is pc-monotone; DVE/ACT
       have a hardware output-drain between ops), so a wait on the
       engine's own completion semaphore is redundant.
    2) Split any remaining multi-wait onto same-engine NoOp instructions
       inserted just before the instruction.
    """
    for f in nc.m.functions:
        for bb in f.blocks:
            out = []
            for inst in bb.instructions:
                si = inst.sync_info
                if si is None or len(si.on_wait) <= 1:
                    out.append(inst)
                    continue
                eng = str(inst.engine).split(".")[-1]
                pfx = _ENG_PREFIX.get(eng)
                waits = list(si.on_wait)
                keep = [
                    w for w in waits
                    if not (pfx and (w.ant_name or "").startswith(pfx))
                ]
                for w in keep[:-1]:
                    nop = mybir.InstNoOp(name=nc.get_next_instruction_name())
                    nop.engine = inst.engine
                    nop.sync_info = mybir.SyncInfo(on_wait=[w], on_update=[])
                    out.append(nop)
                keep = keep[-1:]
                inst.sync_info = mybir.SyncInfo(
                    on_wait=keep, on_update=list(si.on_update)
                )
                out.append(inst)
            bb.instructions = out


# Parity-class geometry: (eh, ec) -> row taps, col taps, ncols.
# Output padded position: row 1+eh+2u, col 1+ec+2v.
_RTAPS = {0: [(0, 0), (2, 1)], 1: [(1, 1)]}
_CTAPS = {0: [(0, 0), (2, 1)], 1: [(1, 1)]}
# u-chunks per eh for the 3-row-chunk pipelined round
# (padded-row chunks A:1..24, B:25..46, C:47..65/66)
_UCHUNKS = {0: [(0, 12), (12, 11), (23, 10)],   # rows 1+2u
            1: [(0, 12), (12, 11), (23, 9)]}    # rows 2+2u
# H-blur c1/c2/c3 row ranges (inclusive) per chunk
_HROWS = [  # (c1a, c1b, c2a, c2b, c3a, c3b)
    (0, 23, 0, 22, 0, 21),
    (24, 45, 23, 44, 22, 43),
    (46, 65, 45, 64, 44, 63),
]


def build_program() -> bass.Bass:
    nc = bass.Bass()
    xp_d = nc.declare_dram_parameter("xp", [BPC, NCI, P, 34 * 34], BF16, isOutput=False)
    wt_d = nc.declare_dram_parameter("wt", [NCO, NCI, P, 9 * P], BF16, isOutput=False)
    dsc_d = nc.declare_dram_parameter("dsc", [P, BPC * NCO], F32, isOutput=False)
    bsc_d = nc.declare_dram_parameter("bsc", [P, NCO], F32, isOutput=False)
    out_d = nc.declare_dram_parameter("out", [BPC, NCO, P, 64 * 64], F32, isOutput=True)

    with ExitStack() as ctx:
        tc = ctx.enter_context(tile.TileContext(nc))
        consts = ctx.enter_context(tc.tile_pool(name="consts", bufs=1))
        xpool = ctx.enter_context(tc.tile_pool(name="xpool", bufs=1))
        psum = ctx.enter_context(tc.tile_pool(name="psum", bufs=8, space="PSUM"))
        spool = ctx.enter_context(tc.tile_pool(name="spool", bufs=2))
        spool1 = ctx.enter_context(tc.tile_pool(name="spool1", bufs=1))
        opool = ctx.enter_context(tc.tile_pool(name="opool", bufs=2))

        w_sb = consts.tile([P, NCO, NCI, 9 * P], BF16, tag="wsb")
        d_sb = consts.tile([P, BPC * NCO], F32, tag="dsb")
        b_sb = consts.tile([P, NCO], F32, tag="bsb")
        x_tiles = {}

        def load_x(s, c):
            t = xpool.tile([P, 34, 34], BF16, tag=f"x{s}{c}", name=f"x{s}{c}")
            nc.sync.dma_start(
                out=t[:], in_=xp_d[s, c].rearrange("p (a b) -> p a b", b=34)
            )
            x_tiles[(s, c)] = t

        # DMA issue order = need order: scales (tiny), then round-0 gate
        # (wt[0,c] + x(0,c) interleaved), then remaining weights, then s=1 x.
        nc.sync.dma_start(out=d_sb[:], in_=dsc_d[:])
        nc.sync.dma_start(out=b_sb[:], in_=bsc_d[:])
        for c in range(NCI):
            nc.sync.dma_start(out=w_sb[:, 0, c, :], in_=wt_d[0, c])
            load_x(0, c)
        for c in range(NCI):
            nc.sync.dma_start(out=w_sb[:, 1, c, :], in_=wt_d[1, c])
        for c in range(NCI):
            load_x(1, c)
        for oc in (2, 3):
            for c in range(NCI):
                nc.sync.dma_start(out=w_sb[:, oc, c, :], in_=wt_d[oc, c])

        # PE warm-up on a zeroed tile: ramps the P-state during the DMA
        # head so real matmuls start at full clock. The memzero is FIRST
        # on ACT (no DMA dependency) so the warm-up starts immediately.
        wz = consts.tile([P, 512], BF16, tag="wz")
        nc.scalar.memzero(wz[:])
        warm_ps = [psum.tile([P, 512], F32, tag="ps", name=f"wps{i}") for i in range(2)]
        for i in range(10):
            nc.tensor.matmul(
                warm_ps[i % 2][:], wz[:, 0:P], wz[:], start=True, stop=True)

        # Persistent column-parity planes of the zero-padded 67x67 grid,
        # stored flat ([67*34] + one pad row so shifted reads stay in
        # bounds). yE col m <-> padded col 2m ; yO col m <-> padded col
        # 2m+1 (col 33 = pad). Zeroed once; borders/pads stay zero,
        # interiors are fully overwritten by every eviction round.
        plane_sets = []
        for i in range(2):
            ye = consts.tile([P, PL + PW], BF16, tag=f"ye{i}")
            yo = consts.tile([P, PL + PW], BF16, tag=f"yo{i}")
            for t in (ye, yo):
                nc.scalar.memzero(t[:])
            plane_sets.append((ye, yo))

        # Engine warm-up ops that absorb DMA-completion waits, so downstream
        # compute instructions stay within the 2-sem-wait ISA limit.
        warm_a = consts.tile([P, 1], F32, tag="warm_a")
        nc.scalar.copy(warm_a[:], d_sb[:, 0:1])
        warm_v = consts.tile([P, 1], F32, tag="warm_v")
        nc.vector.tensor_copy(warm_v[:], b_sb[:, 0:1])

        NR = BPC * NCO  # 8 rounds
        pending = []    # deferred interleave+DMA closures (one round behind)

        def emit_round_full(s, oc, rnd):
            """Rounds 0..NR-2: c-outer matmuls, full-plane blur."""
            yE, yO = plane_sets[rnd % 2]
            for eh, ec in ((0, 0), (0, 1), (1, 0), (1, 1)):
                rtaps, ctaps = _RTAPS[eh], _CTAPS[ec]
                ncols = 33 if ec == 0 else 32
                if eh == 0:
                    rchunks = [(0, 11), (11, 11), (22, 11)]
                elif ec == 0:
                    rchunks = [(0, 11), (11, 11), (22, 10)]
                else:
                    rchunks = [(0, 16), (16, 16)]
                taps = [(kh, kw, ra, cb) for (kh, ra) in rtaps for (kw, cb) in ctaps]
                ptiles = [
                    psum.tile([P, 512], F32, tag="ps", name=f"ps{s}{oc}{eh}{ec}{fc}")
                    for fc in range(len(rchunks))
                ]
                nmm = len(taps) * NCI
                i = 0
                for c in range(NCI):          # c-outer: chunk-0 DMAs gate less
                    for kh, kw, ra, cb in taps:
                        lhsT = w_sb[:, oc, c, (kh * 3 + kw) * P : (kh * 3 + kw + 1) * P]
                        for fc, (u0, nr) in enumerate(rchunks):
                            rhs = x_tiles[(s, c)][:, u0 + ra : u0 + ra + nr,
                                                  cb : cb + ncols]
                            nc.tensor.matmul(
                                ptiles[fc][:, : nr * ncols], lhsT, rhs,
                                start=(i == 0), stop=(i == nmm - 1),
                            )
                        i += 1
                # evict into the parity plane: padded row 1+eh+2u,
                # padded col 1+ec+2v -> ec=0: yO col v ; ec=1: yE col v+1
                plane = yO if ec == 0 else yE
                col0 = 0 if ec == 0 else 1
                pv = plane[:, 0:PL].rearrange("p (r c) -> p r c", c=PW)
                for fc, (u0, nr) in enumerate(rchunks):
                    src = ptiles[fc][:, : nr * ncols].rearrange(
                        "p (r c) -> p r c", c=ncols
                    )
                    rsl = slice(1 + eh + 2 * u0, 1 + eh + 2 * (u0 + nr), 2)
                    nc.scalar.activation(
                        pv[:, rsl, col0 : col0 + ncols], src,
                        mybir.ActivationFunctionType.Copy,
                        bias=0.0,
                        scale=d_sb[:, rnd : rnd + 1],
                    )

            # previous round's interleaves: emitted here (after this
            # round's evictions) so ACT never blocks eviction work.
            for f in pending:
                f()
            pending.clear()

            # --- W blur: three [1,1] passes per output col parity.
            s1E = spool.tile([P, PL], BF16, tag="s1E", name=f"s1E{rnd}")
            s1O = spool.tile([P, PL], BF16, tag="s1O", name=f"s1O{rnd}")
            s2E = spool.tile([P, PL], BF16, tag="s2E", name=f"s2E{rnd}")
            s2O = spool.tile([P, PL], BF16, tag="s2O", name=f"s2O{rnd}")
            zzE = spool.tile([P, PL], BF16, tag="zzE", name=f"zzE{rnd}")
            zzO = spool.tile([P, PL], BF16, tag="zzO", name=f"zzO{rnd}")
            yEs = spool1.tile([P, PL], BF16, tag="yEs", name=f"yEs{rnd}")
            s1Es = spool1.tile([P, PL], BF16, tag="s1Es", name=f"s1Es{rnd}")
            s2Es = spool1.tile([P, PL], BF16, tag="s2Es", name=f"s2Es{rnd}")
            nc.sync.dma_start(out=yEs[:], in_=yE[:, 1 : PL + 1])
            nc.vector.tensor_add(s1E[:], yE[:, 0:PL], yO[:, 0:PL])
            nc.vector.tensor_add(s1O[:], yO[:, 0:PL], yEs[:])
            nc.sync.dma_start(out=s1Es[:, 0 : PL - 1], in_=s1E[:, 1:PL])
            nc.vector.tensor_add(s2E[:], s1E[:], s1O[:])
            nc.vector.tensor_add(s2O[:], s1O[:], s1Es[:])
            nc.sync.dma_start(out=s2Es[:, 0 : PL - 1], in_=s2E[:, 1:PL])
            nc.vector.tensor_add(zzE[:], s2E[:], s2O[:])
            nc.vector.tensor_add(zzO[:], s2O[:], s2Es[:])

            # --- H blur per plane: three flat row-shifted passes. O side
            # first so Pool's c2O/c3O (chain-terminal, nothing on DVE
            # waits for them) finish as early as possible.
            of = opool.tile([P, 64, 64], F32, tag="out", name=f"of{rnd}")
            c3s = {}
            # O-side H tiles use DEDICATED tags (bufs=2 -> alternating
            # slots per round): round r's c1O must not reuse the slot the
            # previous round's Pool c2O is still reading, else DVE stalls
            # ~3.5us/round on the slow Pool op (measured).
            for pw_, zp, t1, t2, t3 in (
                (1, zzO, "c1O", "c2O", "c3O"),
                (0, zzE, "s1E", "s2E", "zzE"),
            ):
                tail_eng = nc.gpsimd if (POOL_OFFLOAD and pw_ == 1) else nc.vector
                c1 = spool.tile([P, PL], BF16, tag=t1, name=f"c1_{rnd}{pw_}")
                nc.vector.tensor_add(
                    c1[:, 0 : 66 * PW], zp[:, 0 : 66 * PW], zp[:, PW : PL])
                c2 = spool.tile([P, PL], BF16, tag=t2, name=f"c2_{rnd}{pw_}")
                tail_eng.tensor_add(
                    c2[:, 0 : 65 * PW], c1[:, 0 : 65 * PW], c1[:, PW : 66 * PW])
                c3 = spool.tile([P, PL], BF16, tag=t3, name=f"c3_{rnd}{pw_}")
                tail_eng.tensor_add(
                    c3[:, 0 : 64 * PW], c2[:, 0 : 64 * PW], c2[:, PW : 65 * PW])
                c3s[pw_] = c3

            def do_interleave(rnd=rnd, s=s, oc=oc, of=of, c3s=c3s):
                # col-interleave + bias + fp32 convert on ACT + out DMA.
                # Deferred to after the NEXT round's evictions: the O side
                # waits on Pool's c3O, and ACT's in-order queue must not
                # block the next round's evictions behind that wait (PE
                # would stall on PSUM reuse).
                for pw_ in (0, 1):
                    c3v = c3s[pw_][:, 0 : 64 * PW].rearrange(
                        "p (r c) -> p r c", c=PW)
                    for rh in (0, 1):
                        nc.scalar.activation(
                            of[:, 32 * rh : 32 * (rh + 1), pw_ : 64 : 2],
                            c3v[:, 32 * rh : 32 * (rh + 1), 0:32],
                            mybir.ActivationFunctionType.Identity,
                            bias=b_sb[:, oc : oc + 1], scale=1.0,
                        )
                for rh in (0, 1):
                    nc.sync.dma_start(
                        out=out_d[s, oc, :, 2048 * rh : 2048 * (rh + 1)],
                        in_=of[:, 32 * rh : 32 * (rh + 1), :].rearrange(
                            "p a b -> p (a b)"),
                    )

            pending.append(do_interleave)

        def emit_round_chunked(s, oc, rnd):
            """Last round: 3 row-chunks, blur pipelined into the matmul
            stream so only ~1/3 of the blur trails the PE."""
            yE, yO = plane_sets[rnd % 2]
            pv = {}
            for plane, key in ((yO, 0), (yE, 1)):   # key = ec
                pv[key] = plane[:, 0:PL].rearrange("p (r c) -> p r c", c=PW)

            s1E = spool.tile([P, PL], BF16, tag="s1E", name=f"s1E{rnd}")
            s1O = spool.tile([P, PL], BF16, tag="s1O", name=f"s1O{rnd}")
            s2E = spool.tile([P, PL], BF16, tag="s2E", name=f"s2E{rnd}")
            s2O = spool.tile([P, PL], BF16, tag="s2O", name=f"s2O{rnd}")
            zzE = spool.tile([P, PL], BF16, tag="zzE", name=f"zzE{rnd}")
            zzO = spool.tile([P, PL], BF16, tag="zzO", name=f"zzO{rnd}")
            yEs = spool1.tile([P, PL], BF16, tag="yEs", name=f"yEs{rnd}")
            s1Es = spool1.tile([P, PL], BF16, tag="s1Es", name=f"s1Es{rnd}")
            s2Es = spool1.tile([P, PL], BF16, tag="s2Es", name=f"s2Es{rnd}")
            c1E = spool.tile([P, PL], BF16, tag="s1E", name=f"c1E{rnd}")
            c1O = spool.tile([P, PL], BF16, tag="c1O", name=f"c1O{rnd}")
            c2E = spool.tile([P, PL], BF16, tag="s2E", name=f"c2E{rnd}")
            c2O = spool.tile([P, PL], BF16, tag="c2O", name=f"c2O{rnd}")
            c3E = spool.tile([P, PL], BF16, tag="zzE", name=f"c3E{rnd}")
            c3O = spool.tile([P, PL], BF16, tag="c3O", name=f"c3O{rnd}")
            of = opool.tile([P, 64, 64], F32, tag="out", name=f"of{rnd}")

            # W-chunk flat row ranges (rows of the 67-row padded grid,
            # chunk A includes pad row 0, chunk C pad row 66)
            wrows = [(0, 25), (25, 47), (47, 67)]

            for ck in range(3):
                # --- matmuls for this chunk, all 4 parity classes
                ptiles = {}
                for eh, ec in ((0, 0), (0, 1), (1, 0), (1, 1)):
                    ptiles[(eh, ec)] = psum.tile(
                        [P, 512], F32, tag="ps", name=f"psc{ck}{eh}{ec}")
                cnt = {}
                tot = {}
                for eh, ec in ptiles:
                    tot[(eh, ec)] = len(_RTAPS[eh]) * len(_CTAPS[ec]) * NCI
                    cnt[(eh, ec)] = 0
                for c in range(NCI):
                    for eh, ec in ((0, 0), (0, 1), (1, 0), (1, 1)):
                        u0, nr = _UCHUNKS[eh][ck]
                        ncols = 33 if ec == 0 else 32
                        for kh, ra in _RTAPS[eh]:
                            for kw, cb in _CTAPS[ec]:
                                lhsT = w_sb[:, oc, c,
                                            (kh * 3 + kw) * P : (kh * 3 + kw + 1) * P]
                                rhs = x_tiles[(s, c)][:, u0 + ra : u0 + ra + nr,
                                                      cb : cb + ncols]
                                i = cnt[(eh, ec)]
                                nc.tensor.matmul(
                                    ptiles[(eh, ec)][:, : nr * ncols], lhsT, rhs,
                                    start=(i == 0), stop=(i == tot[(eh, ec)] - 1),
                                )
                                cnt[(eh, ec)] += 1
                # --- evict chunk
                for eh, ec in ((0, 0), (0, 1), (1, 0), (1, 1)):
                    u0, nr = _UCHUNKS[eh][ck]
                    ncols = 33 if ec == 0 else 32
                    col0 = 0 if ec == 0 else 1
                    src = ptiles[(eh, ec)][:, : nr * ncols].rearrange(
                        "p (r c) -> p r c", c=ncols)
                    rsl = slice(1 + eh + 2 * u0, 1 + eh + 2 * (u0 + nr), 2)
                    nc.scalar.activation(
                        pv[ec][:, rsl, col0 : col0 + ncols], src,
                        mybir.ActivationFunctionType.Copy,
                        bias=0.0,
                        scale=d_sb[:, rnd : rnd + 1],
                    )
                if ck == 1:
                    # round-6 interleaves: after chunk B's evictions its
                    # Pool-side c3O is ready, so ACT won't block chunk C.
                    for f in pending:
                        f()
                    pending.clear()
                # --- W blur for this chunk's rows
                r0, r1 = wrows[ck]
                a, b_ = r0 * PW, r1 * PW
                # s1E/s2E are [P, PL] tiles: clamp the +1-shifted source to
                # PL for the last chunk. The one missing tail element only
                # feeds pad col 33 of the O plane, never read downstream.
                e = min(b_ + 1, PL)
                nc.sync.dma_start(out=yEs[:, a:b_], in_=yE[:, a + 1 : b_ + 1])
                nc.vector.tensor_add(s1E[:, a:b_], yE[:, a:b_], yO[:, a:b_])
                nc.vector.tensor_add(s1O[:, a:b_], yO[:, a:b_], yEs[:, a:b_])
                nc.sync.dma_start(out=s1Es[:, a : e - 1], in_=s1E[:, a + 1 : e])
                nc.vector.tensor_add(s2E[:, a:b_], s1E[:, a:b_], s1O[:, a:b_])
                nc.vector.tensor_add(s2O[:, a:b_], s1O[:, a:b_], s1Es[:, a:b_])
                nc.sync.dma_start(out=s2Es[:, a : e - 1], in_=s2E[:, a + 1 : e])
                nc.vector.tensor_add(zzE[:, a:b_], s2E[:, a:b_], s2O[:, a:b_])
                nc.vector.tensor_add(zzO[:, a:b_], s2O[:, a:b_], s2Es[:, a:b_])
                # --- H blur + interleave + out DMA for this chunk
                c1a, c1b, c2a, c2b, c3a, c3b = _HROWS[ck]
                for zp, c1, c2, c3 in ((zzE, c1E, c2E, c3E), (zzO, c1O, c2O, c3O)):
                    nc.vector.tensor_add(
                        c1[:, c1a * PW : (c1b + 1) * PW],
                        zp[:, c1a * PW : (c1b + 1) * PW],
                        zp[:, (c1a + 1) * PW : (c1b + 2) * PW])
                    nc.vector.tensor_add(
                        c2[:, c2a * PW : (c2b + 1) * PW],
                        c1[:, c2a * PW : (c2b + 1) * PW],
                        c1[:, (c2a + 1) * PW : (c2b + 2) * PW])
                    nc.vector.tensor_add(
                        c3[:, c3a * PW : (c3b + 1) * PW],
                        c2[:, c3a * PW : (c3b + 1) * PW],
                        c2[:, (c3a + 1) * PW : (c3b + 2) * PW])
                for pw_, c3 in ((0, c3E), (1, c3O)):
                    c3v = c3[:, 0 : 64 * PW].rearrange("p (r c) -> p r c", c=PW)
                    nc.scalar.activation(
                        of[:, c3a : c3b + 1, pw_ : 64 : 2],
                        c3v[:, c3a : c3b + 1, 0:32],
                        mybir.ActivationFunctionType.Identity,
                        bias=b_sb[:, oc : oc + 1], scale=1.0,
                    )
                nc.sync.dma_start(
                    out=out_d[s, oc, :, 64 * c3a : 64 * (c3b + 1)],
                    in_=of[:, c3a : c3b + 1, :].rearrange("p a b -> p (a b)"),
                )

        for s in range(BPC):
            for oc in range(NCO):
                rnd = s * NCO + oc
                if s == 0 and oc >= 1:
                    # absorb wt[oc,*] DMA sems before the round needs them
                    for c in range(NCI):
                        pwm = psum.tile([P, 512], F32, tag="ps", name=f"pswt{oc}{c}")
                        nc.tensor.matmul(
                            pwm[:, :16], w_sb[:, oc, c, 0:P],
                            x_tiles[(0, c)][:, 0, 0:16],
                            start=True, stop=True,
                        )
                if rnd == 4:
                    # absorb the x(1,*) DMA sems before s=1 rounds
                    for c in range(NCI):
                        pwm = psum.tile([P, 512], F32, tag="ps", name=f"pswm{c}")
                        nc.tensor.matmul(
                            pwm[:, :16], w_sb[:, 0, c, 0:P],
                            x_tiles[(1, c)][:, 0, 0:16],
                            start=True, stop=True,
                        )
                if rnd == NR - 1:
                    emit_round_chunked(s, oc, rnd)
                else:
                    emit_round_full(s, oc, rnd)
    _fix_waits(nc)
    return nc


def make_in_maps(x, w, weight, bias, affine_w, affine_b):
    x = np.asarray(x, np.float32)
    w = np.asarray(w, np.float32)
    weight = np.asarray(weight, np.float32)
    bias = np.asarray(bias, np.float32)
    affine_w = np.asarray(affine_w, np.float32)
    affine_b = np.asarray(affine_b, np.float32)

    s = w @ affine_w.T + affine_b + 1.0  # (B, CIN)
    wsq = (weight.astype(np.float64) ** 2).sum(axis=(2, 3))  # (COUT, CIN)
    d = 1.0 / np.sqrt((s.astype(np.float64) ** 2) @ wsq.T + 1e-8)  # (B, COUT)
    d16 = (d / 16.0).astype(np.float32)

    xp = np.zeros((B, CIN, 34, 34), np.float32)
    xp[:, :, 1:33, 1:33] = x * s[:, :, None, None]
    xp_bf = xp.astype(ml_dtypes.bfloat16).reshape(B, NCI, P, 34 * 34)

    wf = weight[:, :, ::-1, ::-1]  # spatial flip
    # oc-major layout: wt[oc, c, p, (kh*3+kw)*P + m] = wf[oc*P+m, c*P+p, kh, kw]
    wt = np.ascontiguousarray(
        wf.transpose(1, 2, 3, 0)                    # (CIN, 3, 3, COUT)
        .reshape(NCI, P, 9, NCO, P)
        .transpose(3, 0, 1, 2, 4)                   # (NCO, NCI, P, 9, P)
        .reshape(NCO, NCI, P, 9 * P)
    ).astype(ml_dtypes.bfloat16)

    bsc = np.ascontiguousarray(bias.reshape(COUT).reshape(NCO, P).T).astype(np.float32)

    in_maps = []
    for core in range(NCORES):
        sl = slice(core * BPC, (core + 1) * BPC)
        dcore = d16[sl].reshape(BPC, NCO, P)
        dsc = np.ascontiguousarray(dcore.transpose(2, 0, 1).reshape(P, BPC * NCO))
        in_maps.append(
            {
                "xp": np.ascontiguousarray(xp_bf[sl]),
                "wt": wt,
                "dsc": dsc,
                "bsc": bsc,
            }
        )
    return in_maps


LAST_RESULTS = None  # BassKernelResults of the most recent run (for test harness)


def kernel(x, w, weight, bias, affine_w, affine_b):
    global LAST_RESULTS
    in_maps = make_in_maps(x, w, weight, bias, affine_w, affine_b)
    nc = build_program()
    res = run_bass_kernel_spmd(nc, in_maps, list(range(NCORES)))
    LAST_RESULTS = res
    outs = [r["out"].reshape(BPC, COUT, 64, 64) for r in res.results]
    return np.ascontiguousarray(np.concatenate(outs, axis=0), dtype=np.float32)


# revision 17
# speedup vs baseline: 2.5431x; 1.1934x over previous
"""Trainium2 Bass kernel for nn_Conv2dModulated (modulated transposed conv + blur).

Math restructure (validated vs reference to 5e-7 rel in fp32):
  s = w @ affine_w.T + affine_b + 1                    (B, CIN)  host
  d = rsqrt(s^2 @ sum_kk(W^2).T + 1e-8)               (B, COUT) host
  out[b] = d[b,:]/16 * blur(convT2x(s[b,:] * x[b], W)) + bias
- Modulation folds into x (per-input-channel scale), demodulation into the
  PSUM eviction (per-output-channel scale) -> weights stay sample-independent.
- Stride-2 transposed conv = 4 parity classes of <=2x2-tap convs on the 32x32
  input (subpixel decomposition; 9 effective taps instead of 36 dilated).
- Blur [1,3,3,1]^2/16 = three [1,1] passes per dim. Conv output is kept
  column-parity-split: planes E/O of the zero-padded 67-col grid, stored as
  FLAT [67*34] bf16 rows so every DVE op is one contiguous run (2x mode, no
  per-row bubbles). Shifted operands are SBUF->SBUF DMA copies (free).

Schedule (evolved from the 187us baseline):
- Weights are oc-major [NCO][NCI][P, 9*P] so round 0 gates on just
  wt[0,0]+x(0,0) (~0.6 MB) instead of the whole 5 MB weight block; PE
  warm-up matmuls on a zeroed tile ramp the P-state during the DMA head.
- s1O / c1O blur adds run on Pool (gpsimd) to keep DVE under the 17.3us
  round period (DVE was co-bottleneck with PE at 137us).
- The LAST round's eviction+blur is pipelined in 3 row-chunks so most of
  its blur overlaps its own matmuls (the tail was 41us of trailing DVE).

Sharding: data-parallel over batch, 2 samples per core, 8 cores, no
collectives.
"""

import os
from contextlib import ExitStack

import numpy as np
import ml_dtypes

import concourse.bass as bass
import concourse.tile as tile
from concourse import mybir
from concourse.bass_utils import run_bass_kernel_spmd

B, CIN, COUT, LAT, H, W_SP, KK = 16, 512, 512, 512, 32, 32, 3
NCORES = 8
BPC = B // NCORES  # samples per core
P = 128
NCI = CIN // P
NCO = COUT // P
BF16 = mybir.dt.bfloat16
F32 = mybir.dt.float32
PW = 34          # plane width (67-col padded grid split by col parity)
PL = 67 * PW     # plane flat length (2278)

POOL_OFFLOAD = False  # Pool tensor_add is ~3.8x slower than DVE-2x and every
                      # offload variant measured slower (ring WAR coupling
                      # + ACT queue blocking); keep the whole blur on DVE.

_ENG_PREFIX = {
    "PE": "PE_", "DVE": "DVE_", "Activation": "Activation_",
    "Pool": "Pool_", "SP": "SP_",
}


def _fix_waits(nc: bass.Bass) -> None:
    """Walrus codegen accepts only one sem-wait per compute instruction;
    Tile emits up to 4.

    1) Drop same-engine self-waits: every engine executes its stream
       serially in order (PE matmul completion is pc-monotone; DVE/ACT
       have a hardware output-drain between ops), so a wait on the
       engine's own completion semaphore is redundant.
    2) Split any remaining multi-wait onto same-engine NoOp instructions
       inserted just before the instruction.
    """
    for f in nc.m.functions:
        for bb in f.blocks:
            out = []
            for inst in bb.instructions:
                si = inst.sync_info
                if si is None or len(si.on_wait) <= 1:
                    out.append(inst)
                    continue
                eng = str(inst.engine).split(".")[-1]
                pfx = _ENG_PREFIX.get(eng)
                waits = list(si.on_wait)
                keep = [
                    w for w in waits
                    if not (pfx and (w.ant_name or "").startswith(pfx))
                ]
                for w in keep[:-1]:
                    nop = mybir.InstNoOp(name=nc.get_next_instruction_name())
                    nop.engine = inst.engine
                    nop.sync_info = mybir.SyncInfo(on_wait=[w], on_update=[])
                    out.append(nop)
                keep = keep[-1:]
                inst.sync_info = mybir.SyncInfo(
                    on_wait=keep, on_update=list(si.on_update)
                )
                out.append(inst)
            bb.instructions = out


# Parity-class geometry: (eh, ec) -> row taps, col taps, ncols.
# Output padded position: row 1+eh+2u, col 1+ec+2v.
_RTAPS = {0: [(0, 0), (2, 1)], 1: [(1, 1)]}
_CTAPS = {0: [(0, 0), (2, 1)], 1: [(1, 1)]}
# u-chunks per eh for the 3-row-chunk pipelined round
# (padded-row chunks A:1..24, B:25..46, C:47..65/66)
_UCHUNKS = {0: [(0, 12), (12, 11), (23, 10)],   # rows 1+2u
            1: [(0, 12), (12, 11), (23, 9)]}    # rows 2+2u
# H-blur c1/c2/c3 row ranges (inclusive) per chunk
_HROWS = [  # (c1a, c1b, c2a, c2b, c3a, c3b)
    (0, 23, 0, 22, 0, 21),
    (24, 45, 23, 44, 22, 43),
    (46, 65, 45, 64, 44, 63),
]


def build_program() -> bass.Bass:
    nc = bass.Bass()
    xp_d = nc.declare_dram_parameter("xp", [BPC, NCI, P, 34 * 34], BF16, isOutput=False)
    wt_d = nc.declare_dram_parameter("wt", [NCO, NCI, P, 9 * P], BF16, isOutput=False)
    dsc_d = nc.declare_dram_parameter("dsc", [P, BPC * NCO], F32, isOutput=False)
    bsc_d = nc.declare_dram_parameter("bsc", [P, NCO], F32, isOutput=False)
    out_d = nc.declare_dram_parameter("out", [BPC, NCO, P, 64 * 64], F32, isOutput=True)

    with ExitStack() as ctx:
        tc = ctx.enter_context(tile.TileContext(nc))
        consts = ctx.enter_context(tc.tile_pool(name="consts", bufs=1))
        xpool = ctx.enter_context(tc.tile_pool(name="xpool", bufs=1))
        psum = ctx.enter_context(tc.tile_pool(name="psum", bufs=8, space="PSUM"))
        spool = ctx.enter_context(tc.tile_pool(name="spool", bufs=2))
        spool1 = ctx.enter_context(tc.tile_pool(name="spool1", bufs=1))
        opool = ctx.enter_context(tc.tile_pool(name="opool", bufs=2))

        w_sb = consts.tile([P, NCO, NCI, 9 * P], BF16, tag="wsb")
        d_sb = consts.tile([P, BPC * NCO], F32, tag="dsb")
        b_sb = consts.tile([P, NCO], F32, tag="bsb")
        x_tiles = {}

        def load_x(s, c):
            t = xpool.tile([P, 34, 34], BF16, tag=f"x{s}{c}", name=f"x{s}{c}")
            nc.sync.dma_start(
                out=t[:], in_=xp_d[s, c].rearrange("p (a b) -> p a b", b=34)
            )
            x_tiles[(s, c)] = t

        # DMA issue order = need order: scales (tiny), then round-0 gate
        # (wt[0,c] + x(0,c) interleaved), then remaining weights, then s=1 x.
        nc.sync.dma_start(out=d_sb[:], in_=dsc_d[:])
        nc.sync.dma_start(out=b_sb[:], in_=bsc_d[:])
        for c in range(NCI):
            nc.sync.dma_start(out=w_sb[:, 0, c, :], in_=wt_d[0, c])
            load_x(0, c)
        for c in range(NCI):
            nc.sync.dma_start(out=w_sb[:, 1, c, :], in_=wt_d[1, c])
        for c in range(NCI):
            load_x(1, c)
        for oc in (2, 3):
            for c in range(NCI):
                nc.sync.dma_start(out=w_sb[:, oc, c, :], in_=wt_d[oc, c])

        # PE warm-up on a zeroed tile: ramps the P-state during the DMA
        # head so real matmuls start at full clock. The memzero is FIRST
        # on ACT (no DMA dependency) so the warm-up starts immediately.
        wz = consts.tile([P, 512], BF16, tag="wz")
        nc.scalar.memzero(wz[:])
        warm_ps = [psum.tile([P, 512], F32, tag="ps", name=f"wps{i}") for i in range(2)]
        for i in range(10):
            nc.tensor.matmul(
                warm_ps[i % 2][:], wz[:, 0:P], wz[:], start=True, stop=True)

        # Persistent column-parity planes of the zero-padded 67x67 grid,
        # stored flat ([67*34] + one pad row so shifted reads stay in
        # bounds). yE col m <-> padded col 2m ; yO col m <-> padded col
        # 2m+1 (col 33 = pad). Zeroed once; borders/pads stay zero,
        # interiors are fully overwritten by every eviction round.
        plane_sets = []
        for i in range(2):
            ye = consts.tile([P, PL + PW], BF16, tag=f"ye{i}")
            yo = consts.tile([P, PL + PW], BF16, tag=f"yo{i}")
            for t in (ye, yo):
                nc.scalar.memzero(t[:])
            plane_sets.append((ye, yo))

        # Engine warm-up ops that absorb DMA-completion waits, so downstream
        # compute instructions stay within the 2-sem-wait ISA limit.
        warm_a = consts.tile([P, 1], F32, tag="warm_a")
        nc.scalar.copy(warm_a[:], d_sb[:, 0:1])
        warm_v = consts.tile([P, 1], F32, tag="warm_v")
        nc.vector.tensor_copy(warm_v[:], b_sb[:, 0:1])

        NR = BPC * NCO  # 8 rounds
        pending = []    # deferred interleave+DMA closures (one round behind)

        def emit_round_full(s, oc, rnd):
            """Rounds 0..NR-2: c-outer matmuls, full-plane blur."""
            yE, yO = plane_sets[rnd % 2]
            for eh, ec in ((0, 0), (0, 1), (1, 0), (1, 1)):
                rtaps, ctaps = _RTAPS[eh], _CTAPS[ec]
                ncols = 33 if ec == 0 else 32
                if eh == 0:
                    rchunks = [(0, 11), (11, 11), (22, 11)]
                elif ec == 0:
                    rchunks = [(0, 11), (11, 11), (22, 10)]
                else:
                    rchunks = [(0, 16), (16, 16)]
                taps = [(kh, kw, ra, cb) for (kh, ra) in rtaps for (kw, cb) in ctaps]
                ptiles = [
                    psum.tile([P, 512], F32, tag="ps", name=f"ps{s}{oc}{eh}{ec}{fc}")
                    for fc in range(len(rchunks))
                ]
                nmm = len(taps) * NCI
                i = 0
                for c in range(NCI):          # c-outer: chunk-0 DMAs gate less
                    for kh, kw, ra, cb in taps:
                        lhsT = w_sb[:, oc, c, (kh * 3 + kw) * P : (kh * 3 + kw + 1) * P]
                        for fc, (u0, nr) in enumerate(rchunks):
                            rhs = x_tiles[(s, c)][:, u0 + ra : u0 + ra + nr,
                                                  cb : cb + ncols]
                            nc.tensor.matmul(
                                ptiles[fc][:, : nr * ncols], lhsT, rhs,
                                start=(i == 0), stop=(i == nmm - 1),
                            )
                        i += 1
                # evict into the parity plane: padded row 1+eh+2u,
                # padded col 1+ec+2v -> ec=0: yO col v ; ec=1: yE col v+1
                plane = yO if ec == 0 else yE
                col0 = 0 if ec == 0 else 1
                pv = plane[:, 0:PL].rearrange("p (r c) -> p r c", c=PW)
                for fc, (u0, nr) in enumerate(rchunks):
                    src = ptiles[fc][:, : nr * ncols].rearrange(
                        "p (r c) -> p r c", c=ncols
                    )
                    rsl = slice(1 + eh + 2 * u0, 1 + eh + 2 * (u0 + nr), 2)
                    nc.scalar.activation(
                        pv[:, rsl, col0 : col0 + ncols], src,
                        mybir.ActivationFunctionType.Copy,
                        bias=0.0,
                        scale=d_sb[:, rnd : rnd + 1],
                    )

            # previous round's interleaves: emitted here (after this
            # round's evictions) so ACT never blocks eviction work.
            for f in pending:
                f()
            pending.clear()

            # --- W blur: three [1,1] passes per output col parity.
            s1E = spool.tile([P, PL], BF16, tag="s1E", name=f"s1E{rnd}")
            s1O = spool.tile([P, PL], BF16, tag="s1O", name=f"s1O{rnd}")
            s2E = spool.tile([P, PL], BF16, tag="s2E", name=f"s2E{rnd}")
            s2O = spool.tile([P, PL], BF16, tag="s2O", name=f"s2O{rnd}")
            zzE = spool.tile([P, PL], BF16, tag="zzE", name=f"zzE{rnd}")
            zzO = spool.tile([P, PL], BF16, tag="zzO", name=f"zzO{rnd}")
            yEs = spool1.tile([P, PL], BF16, tag="yEs", name=f"yEs{rnd}")
            s1Es = spool1.tile([P, PL], BF16, tag="s1Es", name=f"s1Es{rnd}")
            s2Es = spool1.tile([P, PL], BF16, tag="s2Es", name=f"s2Es{rnd}")
            nc.sync.dma_start(out=yEs[:], in_=yE[:, 1 : PL + 1])
            nc.vector.tensor_add(s1E[:], yE[:, 0:PL], yO[:, 0:PL])
            nc.vector.tensor_add(s1O[:], yO[:, 0:PL], yEs[:])
            nc.sync.dma_start(out=s1Es[:, 0 : PL - 1], in_=s1E[:, 1:PL])
            nc.vector.tensor_add(s2E[:], s1E[:], s1O[:])
            nc.vector.tensor_add(s2O[:], s1O[:], s1Es[:])
            nc.sync.dma_start(out=s2Es[:, 0 : PL - 1], in_=s2E[:, 1:PL])
            nc.vector.tensor_add(zzE[:], s2E[:], s2O[:])
            nc.vector.tensor_add(zzO[:], s2O[:], s2Es[:])

            # --- H blur per plane: three flat row-shifted passes. O side
            # first so Pool's c2O/c3O (chain-terminal, nothing on DVE
            # waits for them) finish as early as possible.
            of = opool.tile([P, 64, 64], F32, tag="out", name=f"of{rnd}")
            c3s = {}
            # O-side H tiles use DEDICATED tags (bufs=2 -> alternating
            # slots per round): round r's c1O must not reuse the slot the
            # previous round's Pool c2O is still reading, else DVE stalls
            # ~3.5us/round on the slow Pool op (measured).
            for pw_, zp, t1, t2, t3 in (
                (1, zzO, "c1O", "c2O", "c3O"),
                (0, zzE, "s1E", "s2E", "zzE"),
            ):
                tail_eng = nc.gpsimd if (POOL_OFFLOAD and pw_ == 1) else nc.vector
                c1 = spool.tile([P, PL], BF16, tag=t1, name=f"c1_{rnd}{pw_}")
                nc.vector.tensor_add(
                    c1[:, 0 : 66 * PW], zp[:, 0 : 66 * PW], zp[:, PW : PL])
                c2 = spool.tile([P, PL], BF16, tag=t2, name=f"c2_{rnd}{pw_}")
                tail_eng.tensor_add(
                    c2[:, 0 : 65 * PW], c1[:, 0 : 65 * PW], c1[:, PW : 66 * PW])
                c3 = spool.tile([P, PL], BF16, tag=t3, name=f"c3_{rnd}{pw_}")
                tail_eng.tensor_add(
                    c3[:, 0 : 64 * PW], c2[:, 0 : 64 * PW], c2[:, PW : 65 * PW])
                c3s[pw_] = c3

            def do_interleave(rnd=rnd, s=s, oc=oc, of=of, c3s=c3s):
                # col-interleave + bias + fp32 convert on ACT + out DMA.
                # Deferred to after the NEXT round's evictions: the O side
                # waits on Pool's c3O, and ACT's in-order queue must not
                # block the next round's evictions behind that wait (PE
                # would stall on PSUM reuse).
                for pw_ in (0, 1):
                    c3v = c3s[pw_][:, 0 : 64 * PW].rearrange(
                        "p (r c) -> p r c", c=PW)
                    for rh in (0, 1):
                        nc.scalar.activation(
                            of[:, 32 * rh : 32 * (rh + 1), pw_ : 64 : 2],
                            c3v[:, 32 * rh : 32 * (rh + 1), 0:32],
                            mybir.ActivationFunctionType.Identity,
                            bias=b_sb[:, oc : oc + 1], scale=1.0,
                        )
                for rh in (0, 1):
                    nc.sync.dma_start(
                        out=out_d[s, oc, :, 2048 * rh : 2048 * (rh + 1)],
                        in_=of[:, 32 * rh : 32 * (rh + 1), :].rearrange(
                            "p a b -> p (a b)"),
                    )

            pending.append(do_interleave)

        def emit_round_chunked(s, oc, rnd):
            """Last round: 3 row-chunks, blur pipelined into the matmul
            stream so only ~1/3 of the blur trails the PE."""
            yE, yO = plane_sets[rnd % 2]
            pv = {}
            for plane, key in ((yO, 0), (yE, 1)):   # key = ec
                pv[key] = plane[:, 0:PL].rearrange("p (r c) -> p r c", c=PW)

            s1E = spool.tile([P, PL], BF16, tag="s1E", name=f"s1E{rnd}")
            s1O = spool.tile([P, PL], BF16, tag="s1O", name=f"s1O{rnd}")
            s2E = spool.tile([P, PL], BF16, tag="s2E", name=f"s2E{rnd}")
            s2O = spool.tile([P, PL], BF16, tag="s2O", name=f"s2O{rnd}")
            zzE = spool.tile([P, PL], BF16, tag="zzE", name=f"zzE{rnd}")
            zzO = spool.tile([P, PL], BF16, tag="zzO", name=f"zzO{rnd}")
            yEs = spool1.tile([P, PL], BF16, tag="yEs", name=f"yEs{rnd}")
            s1Es = spool1.tile([P, PL], BF16, tag="s1Es", name=f"s1Es{rnd}")
            s2Es = spool1.tile([P, PL], BF16, tag="s2Es", name=f"s2Es{rnd}")
            c1E = spool.tile([P, PL], BF16, tag="s1E", name=f"c1E{rnd}")
            c1O = spool.tile([P, PL], BF16, tag="c1O", name=f"c1O{rnd}")
            c2E = spool.tile([P, PL], BF16, tag="s2E", name=f"c2E{rnd}")
            c2O = spool.tile([P, PL], BF16, tag="c2O", name=f"c2O{rnd}")
            c3E = spool.tile([P, PL], BF16, tag="zzE", name=f"c3E{rnd}")
            c3O = spool.tile([P, PL], BF16, tag="c3O", name=f"c3O{rnd}")
            of = opool.tile([P, 64, 64], F32, tag="out", name=f"of{rnd}")

            # W-chunk flat row ranges (rows of the 67-row padded grid,
            # chunk A includes pad row 0, chunk C pad row 66)
            wrows = [(0, 25), (25, 47), (47, 67)]

            for ck in range(3):
                # --- matmuls for this chunk, all 4 parity classes
                ptiles = {}
                for eh, ec in ((0, 0), (0, 1), (1, 0), (1, 1)):
                    ptiles[(eh, ec)] = psum.tile(
                        [P, 512], F32, tag="ps", name=f"psc{ck}{eh}{ec}")
                cnt = {}
                tot = {}
                for eh, ec in ptiles:
                    tot[(eh, ec)] = len(_RTAPS[eh]) * len(_CTAPS[ec]) * NCI
                    cnt[(eh, ec)] = 0
                for c in range(NCI):
                    for eh, ec in ((0, 0), (0, 1), (1, 0), (1, 1)):
                        u0, nr = _UCHUNKS[eh][ck]
                        ncols = 33 if ec == 0 else 32
                        for kh, ra in _RTAPS[eh]:
                            for kw, cb in _CTAPS[ec]:
                                lhsT = w_sb[:, oc, c,
                                            (kh * 3 + kw) * P : (kh * 3 + kw + 1) * P]
                                rhs = x_tiles[(s, c)][:, u0 + ra : u0 + ra + nr,
                                                      cb : cb + ncols]
                                i = cnt[(eh, ec)]
                                nc.tensor.matmul(
                                    ptiles[(eh, ec)][:, : nr * ncols], lhsT, rhs,
                                    start=(i == 0), stop=(i == tot[(eh, ec)] - 1),
                                )
                                cnt[(eh, ec)] += 1
                # --- evict chunk
                for eh, ec in ((0, 0), (0, 1), (1, 0), (1, 1)):
                    u0, nr = _UCHUNKS[eh][ck]
                    ncols = 33 if ec == 0 else 32
                    col0 = 0 if ec == 0 else 1
                    src = ptiles[(eh, ec)][:, : nr * ncols].rearrange(
                        "p (r c) -> p r c", c=ncols)
                    rsl = slice(1 + eh + 2 * u0, 1 + eh + 2 * (u0 + nr), 2)
                    nc.scalar.activation(
                        pv[ec][:, rsl, col0 : col0 + ncols], src,
                        mybir.ActivationFunctionType.Copy,
                        bias=0.0,
                        scale=d_sb[:, rnd : rnd + 1],
                    )
                if ck == 1:
                    # round-6 interleaves: after chunk B's evictions its
                    # Pool-side c3O is ready, so ACT won't block chunk C.
                    for f in pending:
                        f()
                    pending.clear()
                # --- W blur for this chunk's rows
                r0, r1 = wrows[ck]
                a, b_ = r0 * PW, r1 * PW
                # s1E/s2E are [P, PL] tiles: clamp the +1-shifted source to
                # PL for the last chunk. The one missing tail element only
                # feeds pad col 33 of the O plane, never read downstream.
                e = min(b_ + 1, PL)
                nc.sync.dma_start(out=yEs[:, a:b_], in_=yE[:, a + 1 : b_ + 1])
                nc.vector.tensor_add(s1E[:, a:b_], yE[:, a:b_], yO[:, a:b_])
                nc.vector.tensor_add(s1O[:, a:b_], yO[:, a:b_], yEs[:, a:b_])
                nc.sync.dma_start(out=s1Es[:, a : e - 1], in_=s1E[:, a + 1 : e])
                nc.vector.tensor_add(s2E[:, a:b_], s1E[:, a:b_], s1O[:, a:b_])
                nc.vector.tensor_add(s2O[:, a:b_], s1O[:, a:b_], s1Es[:, a:b_])
                nc.sync.dma_start(out=s2Es[:, a : e - 1], in_=s2E[:, a + 1 : e])
                nc.vector.tensor_add(zzE[:, a:b_], s2E[:, a:b_], s2O[:, a:b_])
                nc.vector.tensor_add(zzO[:, a:b_], s2O[:, a:b_], s2Es[:, a:b_])
                # --- H blur + interleave + out DMA for this chunk
                c1a, c1b, c2a, c2b, c3a, c3b = _HROWS[ck]
                for zp, c1, c2, c3 in ((zzE, c1E, c2E, c3E), (zzO, c1O, c2O, c3O)):
                    nc.vector.tensor_add(
                        c1[:, c1a * PW : (c1b + 1) * PW],
                        zp[:, c1a * PW : (c1b + 1) * PW],
                        zp[:, (c1a + 1) * PW : (c1b + 2) * PW])
                    nc.vector.tensor_add(
                        c2[:, c2a * PW : (c2b + 1) * PW],
                        c1[:, c2a * PW : (c2b + 1) * PW],
                        c1[:, (c2a + 1) * PW : (c2b + 2) * PW])
                    nc.vector.tensor_add(
                        c3[:, c3a * PW : (c3b + 1) * PW],
                        c2[:, c3a * PW : (c3b + 1) * PW],
                        c2[:, (c3a + 1) * PW : (c3b + 2) * PW])
                for pw_, c3 in ((0, c3E), (1, c3O)):
                    c3v = c3[:, 0 : 64 * PW].rearrange("p (r c) -> p r c", c=PW)
                    nc.scalar.activation(
                        of[:, c3a : c3b + 1, pw_ : 64 : 2],
                        c3v[:, c3a : c3b + 1, 0:32],
                        mybir.ActivationFunctionType.Identity,
                        bias=b_sb[:, oc : oc + 1], scale=1.0,
                    )
                nc.sync.dma_start(
                    out=out_d[s, oc, :, 64 * c3a : 64 * (c3b + 1)],
                    in_=of[:, c3a : c3b + 1, :].rearrange("p a b -> p (a b)"),
                )

        for s in range(BPC):
            for oc in range(NCO):
                rnd = s * NCO + oc
                if s == 0 and oc >= 1:
                    # absorb wt[oc,*] DMA sems before the round needs them
                    for c in range(NCI):
                        pwm = psum.tile([P, 512], F32, tag="ps", name=f"pswt{oc}{c}")
                        nc.tensor.matmul(
                            pwm[:, :16], w_sb[:, oc, c, 0:P],
                            x_tiles[(0, c)][:, 0, 0:16],
                            start=True, stop=True,
                        )
                if rnd == 4:
                    # absorb the x(1,*) DMA sems before s=1 rounds
                    for c in range(NCI):
                        pwm = psum.tile([P, 512], F32, tag="ps", name=f"pswm{c}")
                        nc.tensor.matmul(
                            pwm[:, :16], w_sb[:, 0, c, 0:P],
                            x_tiles[(1, c)][:, 0, 0:16],
                            start=True, stop=True,
                        )
                if rnd == NR - 1:
                    emit_round_chunked(s, oc, rnd)
                else:
                    emit_round_full(s, oc, rnd)
    _fix_waits(nc)
    return nc


def make_in_maps(x, w, weight, bias, affine_w, affine_b):
    x = np.asarray(x, np.float32)
    w = np.asarray(w, np.float32)
    weight = np.asarray(weight, np.float32)
    bias = np.asarray(bias, np.float32)
    affine_w = np.asarray(affine_w, np.float32)
    affine_b = np.asarray(affine_b, np.float32)

    s = w @ affine_w.T + affine_b + 1.0  # (B, CIN)
    wsq = (weight.astype(np.float64) ** 2).sum(axis=(2, 3))  # (COUT, CIN)
    d = 1.0 / np.sqrt((s.astype(np.float64) ** 2) @ wsq.T + 1e-8)  # (B, COUT)
    d16 = (d / 16.0).astype(np.float32)

    xp = np.zeros((B, CIN, 34, 34), np.float32)
    xp[:, :, 1:33, 1:33] = x * s[:, :, None, None]
    xp_bf = xp.astype(ml_dtypes.bfloat16).reshape(B, NCI, P, 34 * 34)

    wf = weight[:, :, ::-1, ::-1]  # spatial flip
    # oc-major layout: wt[oc, c, p, (kh*3+kw)*P + m] = wf[oc*P+m, c*P+p, kh, kw]
    wt = np.ascontiguousarray(
        wf.transpose(1, 2, 3, 0)                    # (CIN, 3, 3, COUT)
        .reshape(NCI, P, 9, NCO, P)
        .transpose(3, 0, 1, 2, 4)                   # (NCO, NCI, P, 9, P)
        .reshape(NCO, NCI, P, 9 * P)
    ).astype(ml_dtypes.bfloat16)

    bsc = np.ascontiguousarray(bias.reshape(COUT).reshape(NCO, P).T).astype(np.float32)

    in_maps = []
    for core in range(NCORES):
        sl = slice(core * BPC, (core + 1) * BPC)
        dcore = d16[sl].reshape(BPC, NCO, P)
        dsc = np.ascontiguousarray(dcore.transpose(2, 0, 1).reshape(P, BPC * NCO))
        in_maps.append(
            {
                "xp": np.ascontiguousarray(xp_bf[sl]),
                "wt": wt,
                "dsc": dsc,
                "bsc": bsc,
            }
        )
    return in_maps


LAST_RESULTS = None  # BassKernelResults of the most recent run (for test harness)


def kernel(x, w, weight, bias, affine_w, affine_b):
    global LAST_RESULTS
    in_maps = make_in_maps(x, w, weight, bias, affine_w, affine_b)
    nc = build_program()
    res = run_bass_kernel_spmd(nc, in_maps, list(range(NCORES)))
    LAST_RESULTS = res
    outs = [r["out"].reshape(BPC, COUT, 64, 64) for r in res.results]
    return np.ascontiguousarray(np.concatenate(outs, axis=0), dtype=np.float32)


# revision 21
# speedup vs baseline: 2.5698x; 1.0105x over previous
"""Trainium2 Bass kernel for nn_Conv2dModulated (modulated transposed conv + blur).

Math restructure (validated vs reference to 5e-7 rel in fp32):
  s = w @ affine_w.T + affine_b + 1                    (B, CIN)  host
  d = rsqrt(s^2 @ sum_kk(W^2).T + 1e-8)               (B, COUT) host
  out[b] = d[b,:]/16 * blur(convT2x(s[b,:] * x[b], W)) + bias
- Modulation folds into x (per-input-channel scale), demodulation into the
  PSUM eviction (per-output-channel scale) -> weights stay sample-independent.
- Stride-2 transposed conv = 4 parity classes of <=2x2-tap convs on the 32x32
  input (subpixel decomposition; 9 effective taps instead of 36 dilated).
- Blur [1,3,3,1]^2/16 = three [1,1] passes per dim. Conv output is kept
  column-parity-split: planes E/O of the zero-padded 67-col grid, stored as
  FLAT [67*34] bf16 rows so every DVE op is one contiguous run (2x mode, no
  per-row bubbles). Shifted operands are SBUF->SBUF DMA copies (free).

Schedule (evolved from the 187us baseline):
- Weights are oc-major [NCO][NCI][P, 9*P] so round 0 gates on just
  wt[0,0]+x(0,0) (~0.6 MB) instead of the whole 5 MB weight block; PE
  warm-up matmuls on a zeroed tile ramp the P-state during the DMA head.
- s1O / c1O blur adds run on Pool (gpsimd) to keep DVE under the 17.3us
  round period (DVE was co-bottleneck with PE at 137us).
- The LAST round's eviction+blur is pipelined in 3 row-chunks so most of
  its blur overlaps its own matmuls (the tail was 41us of trailing DVE).

Sharding: data-parallel over batch, 2 samples per core, 8 cores, no
collectives.
"""

import os
from contextlib import ExitStack

import numpy as np
import ml_dtypes

import concourse.bass as bass
import concourse.tile as tile
from concourse import mybir
from concourse.bass_utils import run_bass_kernel_spmd

B, CIN, COUT, LAT, H, W_SP, KK = 16, 512, 512, 512, 32, 32, 3
NCORES = 8
BPC = B // NCORES  # samples per core
P = 128
NCI = CIN // P
NCO = COUT // P
BF16 = mybir.dt.bfloat16
F32 = mybir.dt.float32
PW = 34          # plane width (67-col padded grid split by col parity)
PL = 67 * PW     # plane flat length (2278)

POOL_OFFLOAD = False  # Pool tensor_add is ~3.8x slower than DVE-2x and every
                      # offload variant measured slower (ring WAR coupling
                      # + ACT queue blocking); keep the whole blur on DVE.

_ENG_PREFIX = {
    "PE": "PE_", "DVE": "DVE_", "Activation": "Activation_",
    "Pool": "Pool_", "SP": "SP_",
}


def _fix_waits(nc: bass.Bass) -> None:
    """Walrus codegen accepts only one sem-wait per compute instruction;
    Tile emits up to 4.

    1) Drop same-engine self-waits: every engine executes its stream
       serially in order (PE matmul completion is pc-monotone; DVE/ACT
       have a hardware output-drain between ops), so a wait on the
       engine's own completion semaphore is redundant.
    2) Split any remaining multi-wait onto same-engine NoOp instructions
       inserted just before the instruction.
    """
    for f in nc.m.functions:
        for bb in f.blocks:
            out = []
            for inst in bb.instructions:
                si = inst.sync_info
                if si is None or len(si.on_wait) <= 1:
                    out.append(inst)
                    continue
                eng = str(inst.engine).split(".")[-1]
                pfx = _ENG_PREFIX.get(eng)
                waits = list(si.on_wait)
                keep = [
                    w for w in waits
                    if not (pfx and (w.ant_name or "").startswith(pfx))
                ]
                for w in keep[:-1]:
                    nop = mybir.InstNoOp(name=nc.get_next_instruction_name())
                    nop.engine = inst.engine
                    nop.sync_info = mybir.SyncInfo(on_wait=[w], on_update=[])
                    out.append(nop)
                keep = keep[-1:]
                inst.sync_info = mybir.SyncInfo(
                    on_wait=keep, on_update=list(si.on_update)
                )
                out.append(inst)
            bb.instructions = out


# Parity-class geometry: (eh, ec) -> row taps, col taps, ncols.
# Output padded position: row 1+eh+2u, col 1+ec+2v.
_RTAPS = {0: [(0, 0), (2, 1)], 1: [(1, 1)]}
_CTAPS = {0: [(0, 0), (2, 1)], 1: [(1, 1)]}
# u-chunks per eh for the 3-row-chunk pipelined round
# (padded-row chunks A:1..24, B:25..46, C:47..65/66)
_UCHUNKS = {0: [(0, 12), (12, 11), (23, 10)],   # rows 1+2u
            1: [(0, 12), (12, 11), (23, 9)]}    # rows 2+2u
# H-blur c1/c2/c3 row ranges (inclusive) per chunk
_HROWS = [  # (c1a, c1b, c2a, c2b, c3a, c3b)
    (0, 23, 0, 22, 0, 21),
    (24, 45, 23, 44, 22, 43),
    (46, 65, 45, 64, 44, 63),
]


def build_program() -> bass.Bass:
    nc = bass.Bass()
    xp_d = nc.declare_dram_parameter("xp", [BPC, NCI, P, 34 * 34], BF16, isOutput=False)
    wt_d = nc.declare_dram_parameter("wt", [NCO, NCI, P, 9 * P], BF16, isOutput=False)
    dsc_d = nc.declare_dram_parameter("dsc", [P, BPC * NCO], F32, isOutput=False)
    bsc_d = nc.declare_dram_parameter("bsc", [P, NCO], F32, isOutput=False)
    # Output in bf16: halves the out-DMA bytes (the host upcasts to fp32;
    # adds ~0.2% rel error on top of the 0.49% bf16-conv error, tolerance
    # is 2e-2). The final round's writes are what the end-of-kernel DMA
    # drain waits on.
    out_d = nc.declare_dram_parameter("out", [BPC, NCO, P, 64 * 64], BF16, isOutput=True)

    with ExitStack() as ctx:
        tc = ctx.enter_context(tile.TileContext(nc))
        consts = ctx.enter_context(tc.tile_pool(name="consts", bufs=1))
        xpool = ctx.enter_context(tc.tile_pool(name="xpool", bufs=1))
        psum = ctx.enter_context(tc.tile_pool(name="psum", bufs=8, space="PSUM"))
        spool = ctx.enter_context(tc.tile_pool(name="spool", bufs=2))
        spool1 = ctx.enter_context(tc.tile_pool(name="spool1", bufs=1))
        opool = ctx.enter_context(tc.tile_pool(name="opool", bufs=2))

        w_sb = consts.tile([P, NCO, NCI, 9 * P], BF16, tag="wsb")
        d_sb = consts.tile([P, BPC * NCO], F32, tag="dsb")
        b_sb = consts.tile([P, NCO], F32, tag="bsb")
        x_tiles = {}

        def load_x(s, c):
            t = xpool.tile([P, 34, 34], BF16, tag=f"x{s}{c}", name=f"x{s}{c}")
            nc.sync.dma_start(
                out=t[:], in_=xp_d[s, c].rearrange("p (a b) -> p a b", b=34)
            )
            x_tiles[(s, c)] = t

        # DMA issue order = need order: scales (tiny), then round-0 gate
        # (wt[0,c] + x(0,c) interleaved), then remaining weights, then s=1 x.
        # The (oc=0, c=0) pieces are split fine-grained (per-tap weights,
        # x row slabs matching the fc chunks) so the first matmuls gate on
        # ~100 KB instead of 590 KB and PE streams behind the DMA.
        nc.sync.dma_start(out=d_sb[:], in_=dsc_d[:])
        nc.sync.dma_start(out=b_sb[:], in_=bsc_d[:])
        x00 = xpool.tile([P, 34, 34], BF16, tag="x00", name="x00")
        x_tiles[(0, 0)] = x00
        x00v = xp_d[0, 0].rearrange("p (a b) -> p a b", b=34)

        def wtap(t):
            nc.sync.dma_start(
                out=w_sb[:, 0, 0, t * P : (t + 1) * P],
                in_=wt_d[0, 0, :, t * P : (t + 1) * P])

        wtap(0)
        nc.sync.dma_start(out=x00[:, 0:12], in_=x00v[:, 0:12])
        for t in (2, 6, 8):
            wtap(t)
        nc.sync.dma_start(out=x00[:, 12:23], in_=x00v[:, 12:23])
        nc.sync.dma_start(out=x00[:, 23:34], in_=x00v[:, 23:34])
        for t in (1, 7, 3, 5, 4):
            wtap(t)
        for c in range(1, NCI):
            nc.sync.dma_start(out=w_sb[:, 0, c, :], in_=wt_d[0, c])
            load_x(0, c)
        for c in range(NCI):
            nc.sync.dma_start(out=w_sb[:, 1, c, :], in_=wt_d[1, c])
        for c in range(NCI):
            load_x(1, c)
        for oc in (2, 3):
            for c in range(NCI):
                nc.sync.dma_start(out=w_sb[:, oc, c, :], in_=wt_d[oc, c])

        # PE warm-up on a zeroed tile: ramps the P-state during the DMA
        # head so real matmuls start at full clock. The memzero is FIRST
        # on ACT (no DMA dependency) so the warm-up starts immediately.
        wz = consts.tile([P, 512], BF16, tag="wz")
        nc.scalar.memzero(wz[:])
        warm_ps = [psum.tile([P, 512], F32, tag="ps", name=f"wps{i}") for i in range(2)]
        for i in range(10):
            nc.tensor.matmul(
                warm_ps[i % 2][:], wz[:, 0:P], wz[:], start=True, stop=True)

        # Persistent column-parity planes of the zero-padded 67x67 grid,
        # stored flat ([67*34] + one pad row so shifted reads stay in
        # bounds). yE col m <-> padded col 2m ; yO col m <-> padded col
        # 2m+1 (col 33 = pad). Zeroed once; borders/pads stay zero,
        # interiors are fully overwritten by every eviction round.
        plane_sets = []
        for i in range(2):
            ye = consts.tile([P, PL + PW], BF16, tag=f"ye{i}")
            yo = consts.tile([P, PL + PW], BF16, tag=f"yo{i}")
            for t in (ye, yo):
                nc.scalar.memzero(t[:])
            plane_sets.append((ye, yo))

        # Engine warm-up ops that absorb DMA-completion waits, so downstream
        # compute instructions stay within the 2-sem-wait ISA limit.
        warm_a = consts.tile([P, 1], F32, tag="warm_a")
        nc.scalar.copy(warm_a[:], d_sb[:, 0:1])
        warm_v = consts.tile([P, 1], F32, tag="warm_v")
        nc.vector.tensor_copy(warm_v[:], b_sb[:, 0:1])

        NR = BPC * NCO  # 8 rounds
        pending = []    # deferred interleave+DMA closures (one round behind)

        def emit_round_full(s, oc, rnd):
            """Rounds 0..NR-2: c-outer matmuls, full-plane blur."""
            yE, yO = plane_sets[rnd % 2]
            for eh, ec in ((0, 0), (0, 1), (1, 0), (1, 1)):
                rtaps, ctaps = _RTAPS[eh], _CTAPS[ec]
                ncols = 33 if ec == 0 else 32
                if eh == 0:
                    rchunks = [(0, 11), (11, 11), (22, 11)]
                elif ec == 0:
                    rchunks = [(0, 11), (11, 11), (22, 10)]
                else:
                    rchunks = [(0, 16), (16, 16)]
                taps = [(kh, kw, ra, cb) for (kh, ra) in rtaps for (kw, cb) in ctaps]
                ptiles = [
                    psum.tile([P, 512], F32, tag="ps", name=f"ps{s}{oc}{eh}{ec}{fc}")
                    for fc in range(len(rchunks))
                ]
                nmm = len(taps) * NCI
                i = 0
                for c in range(NCI):          # c-outer: chunk-0 DMAs gate less
                    for kh, kw, ra, cb in taps:
                        lhsT = w_sb[:, oc, c, (kh * 3 + kw) * P : (kh * 3 + kw + 1) * P]
                        for fc, (u0, nr) in enumerate(rchunks):
                            rhs = x_tiles[(s, c)][:, u0 + ra : u0 + ra + nr,
                                                  cb : cb + ncols]
                            nc.tensor.matmul(
                                ptiles[fc][:, : nr * ncols], lhsT, rhs,
                                start=(i == 0), stop=(i == nmm - 1),
                            )
                        i += 1
                # evict into the parity plane: padded row 1+eh+2u,
                # padded col 1+ec+2v -> ec=0: yO col v ; ec=1: yE col v+1
                plane = yO if ec == 0 else yE
                col0 = 0 if ec == 0 else 1
                pv = plane[:, 0:PL].rearrange("p (r c) -> p r c", c=PW)
                for fc, (u0, nr) in enumerate(rchunks):
                    src = ptiles[fc][:, : nr * ncols].rearrange(
                        "p (r c) -> p r c", c=ncols
                    )
                    rsl = slice(1 + eh + 2 * u0, 1 + eh + 2 * (u0 + nr), 2)
                    nc.scalar.activation(
                        pv[:, rsl, col0 : col0 + ncols], src,
                        mybir.ActivationFunctionType.Copy,
                        bias=0.0,
                        scale=d_sb[:, rnd : rnd + 1],
                    )

            # previous round's interleaves: emitted here (after this
            # round's evictions) so ACT never blocks eviction work.
            for f in pending:
                f()
            pending.clear()

            # --- W blur: three [1,1] passes per output col parity.
            s1E = spool.tile([P, PL], BF16, tag="s1E", name=f"s1E{rnd}")
            s1O = spool.tile([P, PL], BF16, tag="s1O", name=f"s1O{rnd}")
            s2E = spool.tile([P, PL], BF16, tag="s2E", name=f"s2E{rnd}")
            s2O = spool.tile([P, PL], BF16, tag="s2O", name=f"s2O{rnd}")
            zzE = spool.tile([P, PL], BF16, tag="zzE", name=f"zzE{rnd}")
            zzO = spool.tile([P, PL], BF16, tag="zzO", name=f"zzO{rnd}")
            yEs = spool1.tile([P, PL], BF16, tag="yEs", name=f"yEs{rnd}")
            s1Es = spool1.tile([P, PL], BF16, tag="s1Es", name=f"s1Es{rnd}")
            s2Es = spool1.tile([P, PL], BF16, tag="s2Es", name=f"s2Es{rnd}")
            nc.sync.dma_start(out=yEs[:], in_=yE[:, 1 : PL + 1])
            nc.vector.tensor_add(s1E[:], yE[:, 0:PL], yO[:, 0:PL])
            nc.vector.tensor_add(s1O[:], yO[:, 0:PL], yEs[:])
            nc.sync.dma_start(out=s1Es[:, 0 : PL - 1], in_=s1E[:, 1:PL])
            nc.vector.tensor_add(s2E[:], s1E[:], s1O[:])
            nc.vector.tensor_add(s2O[:], s1O[:], s1Es[:])
            nc.sync.dma_start(out=s2Es[:, 0 : PL - 1], in_=s2E[:, 1:PL])
            nc.vector.tensor_add(zzE[:], s2E[:], s2O[:])
            nc.vector.tensor_add(zzO[:], s2O[:], s2Es[:])

            # --- H blur per plane: three flat row-shifted passes. O side
            # first so Pool's c2O/c3O (chain-terminal, nothing on DVE
            # waits for them) finish as early as possible.
            of = opool.tile([P, 64, 64], BF16, tag="out", name=f"of{rnd}")
            c3s = {}
            # O-side H tiles use DEDICATED tags (bufs=2 -> alternating
            # slots per round): round r's c1O must not reuse the slot the
            # previous round's Pool c2O is still reading, else DVE stalls
            # ~3.5us/round on the slow Pool op (measured).
            for pw_, zp, t1, t2, t3 in (
                (1, zzO, "c1O", "c2O", "c3O"),
                (0, zzE, "s1E", "s2E", "zzE"),
            ):
                tail_eng = nc.gpsimd if (POOL_OFFLOAD and pw_ == 1) else nc.vector
                c1 = spool.tile([P, PL], BF16, tag=t1, name=f"c1_{rnd}{pw_}")
                nc.vector.tensor_add(
                    c1[:, 0 : 66 * PW], zp[:, 0 : 66 * PW], zp[:, PW : PL])
                c2 = spool.tile([P, PL], BF16, tag=t2, name=f"c2_{rnd}{pw_}")
                tail_eng.tensor_add(
                    c2[:, 0 : 65 * PW], c1[:, 0 : 65 * PW], c1[:, PW : 66 * PW])
                c3 = spool.tile([P, PL], BF16, tag=t3, name=f"c3_{rnd}{pw_}")
                tail_eng.tensor_add(
                    c3[:, 0 : 64 * PW], c2[:, 0 : 64 * PW], c2[:, PW : 65 * PW])
                c3s[pw_] = c3

            def do_interleave(rnd=rnd, s=s, oc=oc, of=of, c3s=c3s):
                # col-interleave + bias + fp32 convert on ACT + out DMA.
                # Deferred to after the NEXT round's evictions: the O side
                # waits on Pool's c3O, and ACT's in-order queue must not
                # block the next round's evictions behind that wait (PE
                # would stall on PSUM reuse).
                for pw_ in (0, 1):
                    c3v = c3s[pw_][:, 0 : 64 * PW].rearrange(
                        "p (r c) -> p r c", c=PW)
                    for rh in (0, 1):
                        nc.scalar.activation(
                            of[:, 32 * rh : 32 * (rh + 1), pw_ : 64 : 2],
                            c3v[:, 32 * rh : 32 * (rh + 1), 0:32],
                            mybir.ActivationFunctionType.Identity,
                            bias=b_sb[:, oc : oc + 1], scale=1.0,
                        )
                for rh in (0, 1):
                    nc.sync.dma_start(
                        out=out_d[s, oc, :, 2048 * rh : 2048 * (rh + 1)],
                        in_=of[:, 32 * rh : 32 * (rh + 1), :].rearrange(
                            "p a b -> p (a b)"),
                    )

            pending.append(do_interleave)

        def emit_round_chunked(s, oc, rnd):
            """Last round: 3 row-chunks, blur pipelined into the matmul
            stream so only ~1/3 of the blur trails the PE."""
            yE, yO = plane_sets[rnd % 2]
            pv = {}
            for plane, key in ((yO, 0), (yE, 1)):   # key = ec
                pv[key] = plane[:, 0:PL].rearrange("p (r c) -> p r c", c=PW)

            s1E = spool.tile([P, PL], BF16, tag="s1E", name=f"s1E{rnd}")
            s1O = spool.tile([P, PL], BF16, tag="s1O", name=f"s1O{rnd}")
            s2E = spool.tile([P, PL], BF16, tag="s2E", name=f"s2E{rnd}")
            s2O = spool.tile([P, PL], BF16, tag="s2O", name=f"s2O{rnd}")
            zzE = spool.tile([P, PL], BF16, tag="zzE", name=f"zzE{rnd}")
            zzO = spool.tile([P, PL], BF16, tag="zzO", name=f"zzO{rnd}")
            yEs = spool1.tile([P, PL], BF16, tag="yEs", name=f"yEs{rnd}")
            s1Es = spool1.tile([P, PL], BF16, tag="s1Es", name=f"s1Es{rnd}")
            s2Es = spool1.tile([P, PL], BF16, tag="s2Es", name=f"s2Es{rnd}")
            c1E = spool.tile([P, PL], BF16, tag="s1E", name=f"c1E{rnd}")
            c1O = spool.tile([P, PL], BF16, tag="c1O", name=f"c1O{rnd}")
            c2E = spool.tile([P, PL], BF16, tag="s2E", name=f"c2E{rnd}")
            c2O = spool.tile([P, PL], BF16, tag="c2O", name=f"c2O{rnd}")
            c3E = spool.tile([P, PL], BF16, tag="zzE", name=f"c3E{rnd}")
            c3O = spool.tile([P, PL], BF16, tag="c3O", name=f"c3O{rnd}")
            of = opool.tile([P, 64, 64], BF16, tag="out", name=f"of{rnd}")

            # W-chunk flat row ranges (rows of the 67-row padded grid,
            # chunk A includes pad row 0, chunk C pad row 66)
            wrows = [(0, 25), (25, 47), (47, 67)]

            for ck in range(3):
                # --- matmuls for this chunk, all 4 parity classes
                ptiles = {}
                for eh, ec in ((0, 0), (0, 1), (1, 0), (1, 1)):
                    ptiles[(eh, ec)] = psum.tile(
                        [P, 512], F32, tag="ps", name=f"psc{ck}{eh}{ec}")
                cnt = {}
                tot = {}
                for eh, ec in ptiles:
                    tot[(eh, ec)] = len(_RTAPS[eh]) * len(_CTAPS[ec]) * NCI
                    cnt[(eh, ec)] = 0
                for c in range(NCI):
                    for eh, ec in ((0, 0), (0, 1), (1, 0), (1, 1)):
                        u0, nr = _UCHUNKS[eh][ck]
                        ncols = 33 if ec == 0 else 32
                        for kh, ra in _RTAPS[eh]:
                            for kw, cb in _CTAPS[ec]:
                                lhsT = w_sb[:, oc, c,
                                            (kh * 3 + kw) * P : (kh * 3 + kw + 1) * P]
                                rhs = x_tiles[(s, c)][:, u0 + ra : u0 + ra + nr,
                                                      cb : cb + ncols]
                                i = cnt[(eh, ec)]
                                nc.tensor.matmul(
                                    ptiles[(eh, ec)][:, : nr * ncols], lhsT, rhs,
                                    start=(i == 0), stop=(i == tot[(eh, ec)] - 1),
                                )
                                cnt[(eh, ec)] += 1
                # --- evict chunk
                for eh, ec in ((0, 0), (0, 1), (1, 0), (1, 1)):
                    u0, nr = _UCHUNKS[eh][ck]
                    ncols = 33 if ec == 0 else 32
                    col0 = 0 if ec == 0 else 1
                    src = ptiles[(eh, ec)][:, : nr * ncols].rearrange(
                        "p (r c) -> p r c", c=ncols)
                    rsl = slice(1 + eh + 2 * u0, 1 + eh + 2 * (u0 + nr), 2)
                    nc.scalar.activation(
                        pv[ec][:, rsl, col0 : col0 + ncols], src,
                        mybir.ActivationFunctionType.Copy,
                        bias=0.0,
                        scale=d_sb[:, rnd : rnd + 1],
                    )

            # ALL evictions are emitted above, BEFORE any blur/interleave:
            # ACT's in-order queue must never block an eviction behind an
            # interleave that waits on DVE (measured: that serialized the
            # chunks and re-created the 40us tail).
            for f in pending:
                f()
            pending.clear()

            for ck in range(3):
                # --- W blur for this chunk's rows
                r0, r1 = wrows[ck]
                a, b_ = r0 * PW, r1 * PW
                # s1E/s2E are [P, PL] tiles: clamp the +1-shifted source to
                # PL for the last chunk. The one missing tail element only
                # feeds pad col 33 of the O plane, never read downstream.
                e = min(b_ + 1, PL)
                nc.sync.dma_start(out=yEs[:, a:b_], in_=yE[:, a + 1 : b_ + 1])
                nc.vector.tensor_add(s1E[:, a:b_], yE[:, a:b_], yO[:, a:b_])
                nc.vector.tensor_add(s1O[:, a:b_], yO[:, a:b_], yEs[:, a:b_])
                nc.sync.dma_start(out=s1Es[:, a : e - 1], in_=s1E[:, a + 1 : e])
                nc.vector.tensor_add(s2E[:, a:b_], s1E[:, a:b_], s1O[:, a:b_])
                nc.vector.tensor_add(s2O[:, a:b_], s1O[:, a:b_], s1Es[:, a:b_])
                nc.sync.dma_start(out=s2Es[:, a : e - 1], in_=s2E[:, a + 1 : e])
                nc.vector.tensor_add(zzE[:, a:b_], s2E[:, a:b_], s2O[:, a:b_])
                nc.vector.tensor_add(zzO[:, a:b_], s2O[:, a:b_], s2Es[:, a:b_])
                # --- H blur + interleave + out DMA for this chunk
                c1a, c1b, c2a, c2b, c3a, c3b = _HROWS[ck]
                for zp, c1, c2, c3 in ((zzE, c1E, c2E, c3E), (zzO, c1O, c2O, c3O)):
                    nc.vector.tensor_add(
                        c1[:, c1a * PW : (c1b + 1) * PW],
                        zp[:, c1a * PW : (c1b + 1) * PW],
                        zp[:, (c1a + 1) * PW : (c1b + 2) * PW])
                    nc.vector.tensor_add(
                        c2[:, c2a * PW : (c2b + 1) * PW],
                        c1[:, c2a * PW : (c2b + 1) * PW],
                        c1[:, (c2a + 1) * PW : (c2b + 2) * PW])
                    nc.vector.tensor_add(
                        c3[:, c3a * PW : (c3b + 1) * PW],
                        c2[:, c3a * PW : (c3b + 1) * PW],
                        c2[:, (c3a + 1) * PW : (c3b + 2) * PW])
                for pw_, c3 in ((0, c3E), (1, c3O)):
                    c3v = c3[:, 0 : 64 * PW].rearrange("p (r c) -> p r c", c=PW)
                    nc.scalar.activation(
                        of[:, c3a : c3b + 1, pw_ : 64 : 2],
                        c3v[:, c3a : c3b + 1, 0:32],
                        mybir.ActivationFunctionType.Identity,
                        bias=b_sb[:, oc : oc + 1], scale=1.0,
                    )
                nc.sync.dma_start(
                    out=out_d[s, oc, :, 64 * c3a : 64 * (c3b + 1)],
                    in_=of[:, c3a : c3b + 1, :].rearrange("p a b -> p (a b)"),
                )

        for s in range(BPC):
            for oc in range(NCO):
                rnd = s * NCO + oc
                if s == 0 and oc >= 1:
                    # absorb wt[oc,*] DMA sems before the round needs them
                    for c in range(NCI):
                        pwm = psum.tile([P, 512], F32, tag="ps", name=f"pswt{oc}{c}")
                        nc.tensor.matmul(
                            pwm[:, :16], w_sb[:, oc, c, 0:P],
                            x_tiles[(0, c)][:, 0, 0:16],
                            start=True, stop=True,
                        )
                if rnd == 4:
                    # absorb the x(1,*) DMA sems before s=1 rounds
                    for c in range(NCI):
                        pwm = psum.tile([P, 512], F32, tag="ps", name=f"pswm{c}")
                        nc.tensor.matmul(
                            pwm[:, :16], w_sb[:, 0, c, 0:P],
                            x_tiles[(1, c)][:, 0, 0:16],
                            start=True, stop=True,
                        )
                if rnd == NR - 1:
                    emit_round_chunked(s, oc, rnd)
                else:
                    emit_round_full(s, oc, rnd)
    _fix_waits(nc)
    return nc


def make_in_maps(x, w, weight, bias, affine_w, affine_b):
    x = np.asarray(x, np.float32)
    w = np.asarray(w, np.float32)
    weight = np.asarray(weight, np.float32)
    bias = np.asarray(bias, np.float32)
    affine_w = np.asarray(affine_w, np.float32)
    affine_b = np.asarray(affine_b, np.float32)

    s = w @ affine_w.T + affine_b + 1.0  # (B, CIN)
    wsq = (weight.astype(np.float64) ** 2).sum(axis=(2, 3))  # (COUT, CIN)
    d = 1.0 / np.sqrt((s.astype(np.float64) ** 2) @ wsq.T + 1e-8)  # (B, COUT)
    d16 = (d / 16.0).astype(np.float32)

    xp = np.zeros((B, CIN, 34, 34), np.float32)
    xp[:, :, 1:33, 1:33] = x * s[:, :, None, None]
    xp_bf = xp.astype(ml_dtypes.bfloat16).reshape(B, NCI, P, 34 * 34)

    wf = weight[:, :, ::-1, ::-1]  # spatial flip
    # oc-major layout: wt[oc, c, p, (kh*3+kw)*P + m] = wf[oc*P+m, c*P+p, kh, kw]
    wt = np.ascontiguousarray(
        wf.transpose(1, 2, 3, 0)                    # (CIN, 3, 3, COUT)
        .reshape(NCI, P, 9, NCO, P)
        .transpose(3, 0, 1, 2, 4)                   # (NCO, NCI, P, 9, P)
        .reshape(NCO, NCI, P, 9 * P)
    ).astype(ml_dtypes.bfloat16)

    bsc = np.ascontiguousarray(bias.reshape(COUT).reshape(NCO, P).T).astype(np.float32)

    in_maps = []
    for core in range(NCORES):
        sl = slice(core * BPC, (core + 1) * BPC)
        dcore = d16[sl].reshape(BPC, NCO, P)
        dsc = np.ascontiguousarray(dcore.transpose(2, 0, 1).reshape(P, BPC * NCO))
        in_maps.append(
            {
                "xp": np.ascontiguousarray(xp_bf[sl]),
                "wt": wt,
                "dsc": dsc,
                "bsc": bsc,
            }
        )
    return in_maps


LAST_RESULTS = None  # BassKernelResults of the most recent run (for test harness)


def kernel(x, w, weight, bias, affine_w, affine_b):
    global LAST_RESULTS
    in_maps = make_in_maps(x, w, weight, bias, affine_w, affine_b)
    nc = build_program()
    res = run_bass_kernel_spmd(nc, in_maps, list(range(NCORES)))
    LAST_RESULTS = res
    outs = [np.asarray(r["out"], np.float32).reshape(BPC, COUT, 64, 64)
            for r in res.results]
    return np.ascontiguousarray(np.concatenate(outs, axis=0), dtype=np.float32)
